# revision 22
# baseline (speedup 1.0000x reference)
"""MinLSTM Trainium2 kernel (fp8 DoubleRow edition).

Full-input contract: kernel(**inputs) takes the complete (unsharded) numpy
inputs of the reference model and returns the full [B, T+1, H] float32 output.

Math (per batch b, channel h — identical to the reference's log-space scan,
computed in linear space; every quantity is positive so the linear recurrence
is numerically stable):
    a = x @ W_f ;  b = x @ W_i ;  c = x @ W_h          (zero biases folded)
    f = sigmoid(a) / (sigmoid(a) + sigmoid(b))          # forget gate
    i = 1 - f                                           # input gate
    g = max(c + 0.5, sigmoid(c))                        # = exp(log_g(c))
    h_t = f_t * h_{t-1} + i_t * g_t,   h_{-1} = g(h_0)
    out[:, 0] = g(h_0); out[:, t+1] = h_t

Sharding: 8 cores, core c -> (sample b = c//2, H-half hh = c%2, 256 channels).
Fully independent cores, no collectives; host assembles the output.

Device pipeline per 512-wide T-chunk, per 128-channel tile:
  - PE: fp8 e4m3 DoubleRow matmuls (0.5 cyc/row, 2 k-tiles per instruction).
    The f/i gates use plain x8@W8; the c gate adds two error-feedback terms
    (x8@(W-W8)8 + (x-x8)8@W8) which cut the end-to-end quantization error
    from ~2.0e-2 to ~0.9e-2 normwise.
  - Act: one Sigmoid instruction over the 3-gate PSUM tile [128, 3, TC].
  - DVE: fused custom op f = sa*~recip(sa+sb) (7 ALU stages, one pass);
    g = (pc + 0.5) max sg (scalar_tensor_tensor); tensor_tensor_scan.
  - Pool: v = (1-f)*g multiply; most of w = 1-f (ones - f subtract).
  - A fraction of w ops go to Act (Copy, scale=-1, bias=1) to balance.
Outputs stored f16 (DMA halved), upcast on host.
"""

from contextlib import ExitStack

import numpy as np
import ml_dtypes

import concourse.bacc as bacc
import concourse.tile as tile
import concourse.mybir as mybir
from concourse.bass_utils import run_bass_kernel_spmd

import concourse.dve_ops as _dve_ops
from concourse.dve_spec import (Spec as _Spec, Src0 as _S0, Src1 as _S1,
                                C1 as _C1, C2 as _C2,
                                AluOp as _AluOp, Bin as _Bin, lower as _lower)
from concourse.dve_uop import DveOpSpec as _DveOpSpec
from concourse.dve_table_gen import dve_ver_for as _dve_ver_for

# ---- fused custom DVE op: f = in0 * ~1/(in0+in1) ---------------------------
# x = in0+in1; nx = bitcast(~x) (exponent-flip reciprocal seed); u = x*nx
# lands in [-4.5,-4]; f = in0 * nx * (c1*u + c2), deg-1 minimax of 1/u on
# that interval (max rel err ~2.2e-3). 7 ALU stages -> single DVE pass.
GATE_FRAC_CONSTS = {"s1": -0.0546648, "imm2": -0.4681172}


def _register_gate_frac():
    name = "GATE_FRAC_ANT"
    if name in _dve_ops._SUB_OPCODE_FOR_NAME:
        return next(o for o in _dve_ops.OPS if o.name == name)
    _x = _S0 + _S1
    _nx = _Bin(_AluOp.BITWISE_NOT, _x, _x)
    _u = _x * _nx

    def _ref(in0, in1, c0, c1, c2):
        in0 = np.asarray(in0, np.float32)
        in1 = np.asarray(in1, np.float32)
        x = (in0 + in1).astype(np.float32)
        nx = (~x.view(np.int32)).view(np.float32)
        u = x * nx
        return (nx * (c1 * u + c2)) * in0

    spec = _Spec(body=(_Bin(_AluOp.MULTIPLY, _nx, (_C1 * _u + _C2))) * _S0,
                 reference=_ref)
    row = _dve_ops._CUSTOM_DVE_ROW_BASE + len(_dve_ops.OPS)
    assert row < 0x20
    ver = _dve_ver_for("TRN2")
    sha = _DveOpSpec(name=name, opcode=row, uops=_lower(spec, ver=ver),
                     rd1_en=True).sha(ver)
    op = _dve_ops.DveOp(name, spec, subdim=False, uops_sha={ver: sha})
    _dve_ops.OPS.append(op)
    _dve_ops.CUSTOM_DVE_SPECS[name] = spec
    _dve_ops._SUB_OPCODE_FOR_NAME[name] = row
    return op


_GATE_FRAC_OP = _register_gate_frac()

F32 = mybir.dt.float32
F16 = mybir.dt.float16
F8 = mybir.dt.float8e4
AF = mybir.ActivationFunctionType
OP = mybir.AluOpType
PM = mybir.MatmulPerfMode
E4 = ml_dtypes.float8_e4m3fn

B, T, D, H = 4, 8192, 512, 512
NCORES = 8
HS = H // 2          # 256 channels per core
TC = 512             # T chunk width
NCH = T // TC        # 16 chunks
NHT = HS // 128      # 2 h-tiles per core
NPAIR = 2            # DoubleRow k-tile pairs covering D=512

MM_MODE = "fp8dr"    # kept for test.py compatibility

# every W_ACT_EVERY-th (chunk,ht) unit computes w = 1-f on Act instead of
# Pool, balancing Act ~53us / Pool ~60us under the DVE ~63us bottleneck
W_ACT_EVERY = 1   # unused, kept for reference
# w-engine pattern: p=Pool, d=DVE, a=Act (balances the three engines)
W_PATTERN = "ppa"

_nc_cache = {}


def _build_nc(mm_mode=MM_MODE):
    nc = bacc.Bacc("TRN2", target_bir_lowering=False, debug=False,
                   num_devices=NCORES)
    # x8 (slots 0-3) and xlo8 (slots 4-7) k-tiles, chunk-major
    xq = nc.dram_tensor("xq", [NCH * 128, 8, TC], F8, kind="ExternalInput")
    w8 = nc.dram_tensor("w8", [128, NPAIR, 2, 3 * HS], F8,
                        kind="ExternalInput")
    wlo = nc.dram_tensor("wlo", [128, NPAIR, 2, 3 * HS], F8,
                          kind="ExternalInput")
    aux = nc.dram_tensor("aux", [128, NHT], F32, kind="ExternalInput")
    out = nc.dram_tensor("out", [128, NHT, T], F16, kind="ExternalOutput")

    with tile.TileContext(nc) as tc, ExitStack() as ctx:
        wpool = ctx.enter_context(tc.tile_pool(name="w", bufs=1))
        xpool = ctx.enter_context(tc.tile_pool(name="x", bufs=4))
        gpool = ctx.enter_context(tc.tile_pool(name="g", bufs=6))
        hpool = ctx.enter_context(tc.tile_pool(name="h", bufs=5))
        ppool = ctx.enter_context(tc.tile_pool(name="p", bufs=2, space="PSUM"))

        # weight/aux loads on the ACT HWDGE queue so the first x-chunk loads
        # (SP queue) run in parallel with them
        wt = wpool.tile([128, NPAIR, 2, 3 * HS], F8, tag="w8", name="wt")
        nc.scalar.dma_start(wt[:], w8[:])
        wlt = wpool.tile([128, NPAIR, 2, 3 * HS], F8, tag="wlo",
                         name="wlt")
        nc.scalar.dma_start(wlt[:], wlo[:])
        auxt = wpool.tile([128, NHT], F32, tag="aux", name="auxt")
        nc.scalar.dma_start(auxt[:], aux[:])
        ones = wpool.tile([128, TC], F16, tag="ones", name="ones")
        nc.vector.memset(ones[:], 1.0)

        # PE p-state warmup: the tensor engine needs ~3us of continuous work
        # to reach full clock. The first ~5us of the kernel are DMA-only, so
        # burn them on scratch matmuls (no data dependencies) and the first
        # real matmuls run at full speed.
        wu_s = wpool.tile([128, 2, 128], F8, tag="wu_s", name="wu_s")
        nc.vector.memset(wu_s[:], 0.0)
        wu_m = wpool.tile([128, 2, TC], F8, tag="wu_m", name="wu_m")
        nc.vector.memset(wu_m[:], 0.0)
        wu_p = ppool.tile([128, TC], F32, tag="Pc", name="wu_p", bufs=4)
        for _ in range(12):
            nc.tensor.matmul(wu_p[:], wu_s[:], wu_m[:], start=True,
                             stop=True, perf_mode=PM.DoubleRow)

        # chunk 0 split in half so the pipeline primes on half-size units
        chunks = [(0, TC // 2), (TC // 2, TC // 2)]
        chunks += [(k * TC, TC) for k in range(1, NCH)]

        NUNITS = len(chunks) * NHT
        carry = [None] * NHT
        it = 0
        # two-unit software pipeline: unit k's head [matmuls, sigmoid, g, f]
        # is emitted before unit k-2's tail [w, v, scan, out-dma]. The scan
        # of unit k-2 then never stalls the in-order DVE: its v input has
        # been computable since unit k-1's head ran.
        TAIL_DELAY = 2
        from collections import deque
        pendq = deque()  # (f, g, h_slice, ini, carry_ap, ht, ci, tw, unit#)
        out_dmas = {}    # ci -> (ht_tile, tsl, tw) emitted when ht=1 scanned

        def flush_pending(force=False):
            while pendq and (force or len(pendq) > TAIL_DELAY):
                f_, g_, h_, carry_ap, ht_, ci_, tw_, k_ = pendq.popleft()
                # resolve the scan initial lazily: the same-ht predecessor's
                # scan is emitted exactly TAIL_DELAY units before this flush
                ini_ = auxt[:, ht_:ht_ + 1] if ci_ == 0 else carry[ht_]
                # w = 1-f: Pool / DVE / Act per balance pattern; in the
                # drain (Act idle, Pool the laggard) w goes to Act and v to
                # DVE so the last scans issue as early as possible
                drain = k_ >= NUNITS - 3
                w_ = gpool.tile([128, TC], F16, tag="w", name="w_")[:, :tw_]
                wsel = "a" if drain else W_PATTERN[k_ % len(W_PATTERN)]
                if wsel == "a":
                    nc.scalar.activation(w_, f_, AF.Copy, bias=1.0,
                                         scale=-1.0)
                elif wsel == "d":
                    nc.vector.tensor_scalar(w_, f_, -1.0, 1.0,
                                            OP.mult, OP.add)
                else:
                    nc.gpsimd.tensor_tensor(w_, ones[:, :tw_], f_,
                                            op=OP.subtract)
                v_ = gpool.tile([128, TC], F16, tag="v", name="v_")[:, :tw_]
                if drain:
                    nc.vector.tensor_tensor(v_, w_, g_, op=OP.mult)
                else:
                    nc.gpsimd.tensor_tensor(v_, w_, g_, op=OP.mult)
                nc.vector.tensor_tensor_scan(h_, f_, v_, ini_, OP.mult,
                                             OP.add)
                carry[ht_] = carry_ap
                if ci_ == len(chunks) - 1:
                    htile_, tsl_, twc_ = out_dmas[ci_]
                    nc.sync.dma_start(out[:, ht_:ht_ + 1, tsl_],
                                      htile_[:, ht_:ht_ + 1, :twc_])
                elif ht_ == NHT - 1:
                    htile_, tsl_, twc_ = out_dmas.pop(ci_)
                    nc.sync.dma_start(out[:, :, tsl_], htile_[:, :, :twc_])

        for ci, (t0, tw) in enumerate(chunks):
            tsl = slice(t0, t0 + tw)
            xall = xpool.tile([128, 8, TC], F8, tag="xall", name="xall")
            crow = (t0 // TC) * 128
            csl = slice(t0 % TC, t0 % TC + tw)
            nc.sync.dma_start(xall[:, :, csl.start:csl.stop] if tw != TC
                              else xall[:],
                              xq[crow:crow + 128, :, csl])
            ht_tile = hpool.tile([128, NHT, TC], F16, tag="h", name="ht_tile")
            out_dmas[ci] = (ht_tile, tsl, tw)
            for ht in range(NHT):
                # split PSUM: Pc (1 bank, 4-deep rotation — its late reader
                # g never gates the PE) and Pab (2 banks, 2-deep — freed by
                # sigmoid-ab alone)
                Pc = ppool.tile([128, TC], F32, tag="Pc", name="Pc", bufs=4)
                Pab = ppool.tile([128, 2, TC], F32, tag="Pab", name="Pab",
                                 bufs=2)
                for gate in range(3):
                    c0 = gate * HS + ht * 128
                    dst = Pc[:, :tw] if gate == 2 else Pab[:, gate, :tw]
                    nterm = 3 if gate == 2 else 2
                    step = 0
                    for term in range(nterm):
                        for p in range(NPAIR):
                            if term == 1:       # x8 @ Wlo8
                                stat = wlt[:, p, :, c0:c0 + 128]
                                mov = xall[:, 2 * p:2 * p + 2, csl]
                            elif term == 2:     # xlo8 @ W8 (c gate only)
                                stat = wt[:, p, :, c0:c0 + 128]
                                mov = xall[:, 4 + 2 * p:4 + 2 * p + 2, csl]
                            else:               # x8 @ W8
                                stat = wt[:, p, :, c0:c0 + 128]
                                mov = xall[:, 2 * p:2 * p + 2, csl]
                            nc.tensor.matmul(
                                dst, stat, mov,
                                start=(step == 0),
                                stop=(step == 2 * nterm - 1),
                                perf_mode=PM.DoubleRow)
                            step += 1
                # sigmoid-c first (g on DVE consumes it), then sigmoid-ab
                sg = gpool.tile([128, TC], F16, tag="sg", name="sg")[:, :tw]
                nc.scalar.activation(sg, Pc[:, :tw], AF.Sigmoid)
                sab = gpool.tile([128, 2, TC], F16, tag="sab", name="sab")
                nc.scalar.activation(sab[:, :, :tw], Pab[:, :, :tw],
                                     AF.Sigmoid)
                # g = (pc + 0.5) max sg on DVE; last reader of Pc
                g = gpool.tile([128, TC], F16, tag="g", name="g")[:, :tw]
                nc.vector.scalar_tensor_tensor(g, Pc[:, :tw], 0.5, sg,
                                               OP.add, OP.max)
                # f = sa/(sa+sb) fused on DVE
                f = gpool.tile([128, TC], F16, tag="f", name="f")[:, :tw]
                nc.vector._custom_dve(_GATE_FRAC_OP, out=f,
                                      in0=sab[:, 0, :tw], in1=sab[:, 1, :tw],
                                      s0=0.0, s1=GATE_FRAC_CONSTS["s1"],
                                      imm2=GATE_FRAC_CONSTS["imm2"])
                # emit the PREVIOUS unit's tail (w, v, scan, out-dma) now
                # that this unit's head is queued ahead of it
                flush_pending()
                h = ht_tile[:, ht, :tw]
                carry_ap = ht_tile[:, ht, tw - 1:tw]
                pendq.append((f, g, h, carry_ap, ht, ci, tw, it))
                it += 1
        flush_pending(force=True)
    nc.compile()
    return nc


def _get_nc(mm_mode=MM_MODE):
    if mm_mode not in _nc_cache:
        _nc_cache[mm_mode] = _build_nc(mm_mode)
    return _nc_cache[mm_mode]


def _g_host(x):
    # exp(log_g(x)) of the reference, computed directly in fp32
    return np.where(x >= 0, x + 0.5, 1.0 / (1.0 + np.exp(-np.minimum(x, 0))))


def _prep_x(xb):
    """xb [T, D] f32 -> [NCH*128, 8, TC] e4m3 (x8 + xlo8 k-tiles)."""
    xt = np.ascontiguousarray(xb.reshape(NCH, TC, D).transpose(0, 2, 1))
    x8 = xt.astype(E4)                                   # [NCH, D, TC]
    xlo8 = (xt - x8.astype(np.float32)).astype(E4)
    # [NCH, 8, 128, TC] slot-major -> [NCH, 128, 8, TC]
    both = np.concatenate([x8.reshape(NCH, 4, 128, TC),
                           xlo8.reshape(NCH, 4, 128, TC)], axis=1)
    return np.ascontiguousarray(both.transpose(0, 2, 1, 3)).reshape(
        NCH * 128, 8, TC)


def _prep_w(wcat):
    """[D, C] f32 -> ([128, NPAIR, 2, C] e4m3 base, same-shape lo residual
    of the last HS columns)."""
    w8 = wcat.astype(E4)
    dev = np.ascontiguousarray(
        w8.reshape(NPAIR, 2, 128, wcat.shape[1]).transpose(2, 0, 1, 3))
    return dev


def _run(inputs, mm_mode=MM_MODE, trace=False):
    x = np.asarray(inputs["x"], np.float32)
    h_0 = np.asarray(inputs["h_0"], np.float32)
    W_f = np.asarray(inputs["W_f"], np.float32)
    b_f = np.asarray(inputs["b_f"], np.float32)
    W_i = np.asarray(inputs["W_i"], np.float32)
    b_i = np.asarray(inputs["b_i"], np.float32)
    W_h = np.asarray(inputs["W_h"], np.float32)
    b_h = np.asarray(inputs["b_h"], np.float32)
    assert (b_f == 0).all() and (b_i == 0).all() and (b_h == 0).all(), \
        "device program folds zero biases"

    g0 = _g_host(h_0[:, 0, :])  # [B, H]
    xqs = [_prep_x(x[b]) for b in range(B)]

    in_maps = []
    for c in range(NCORES):
        b, hh = divmod(c, 2)
        hs = slice(hh * HS, (hh + 1) * HS)
        wcat = np.concatenate([W_f[:, hs], W_i[:, hs], W_h[:, hs]], axis=1)
        w8dev = _prep_w(wcat)
        wlo_cat = wcat - wcat.astype(E4).astype(np.float32)
        wlodev = np.ascontiguousarray(
            wlo_cat.astype(E4).reshape(NPAIR, 2, 128,
                                       3 * HS).transpose(2, 0, 1, 3))
        auxa = np.ascontiguousarray(
            g0[b, hs].reshape(NHT, 128).T.astype(np.float32))
        in_maps.append({"xq": xqs[b], "w8": w8dev, "wlo": wlodev,
                        "aux": auxa})

    nc = _get_nc(mm_mode)
    res = run_bass_kernel_spmd(nc, in_maps, core_ids=list(range(NCORES)),
                               trace=trace)

    out = np.empty((B, T + 1, H), np.float32)
    out[:, 0, :] = g0
    for c in range(NCORES):
        b, hh = divmod(c, 2)
        hs = slice(hh * HS, (hh + 1) * HS)
        blk = res.results[c]["out"].astype(np.float32)   # [128, NHT, T]
        out[b, 1:, hs] = blk.transpose(2, 1, 0).reshape(T, HS)
    return out, res


def kernel(**inputs):
    out, _ = _run(inputs)
    return out


# revision 23
# speedup vs baseline: 1.0077x; 1.0077x over previous
"""MinLSTM Trainium2 kernel (fp8 DoubleRow edition).

Full-input contract: kernel(**inputs) takes the complete (unsharded) numpy
inputs of the reference model and returns the full [B, T+1, H] float32 output.

Math (per batch b, channel h — identical to the reference's log-space scan,
computed in linear space; every quantity is positive so the linear recurrence
is numerically stable):
    a = x @ W_f ;  b = x @ W_i ;  c = x @ W_h          (zero biases folded)
    f = sigmoid(a) / (sigmoid(a) + sigmoid(b))          # forget gate
    i = 1 - f                                           # input gate
    g = max(c + 0.5, sigmoid(c))                        # = exp(log_g(c))
    h_t = f_t * h_{t-1} + i_t * g_t,   h_{-1} = g(h_0)
    out[:, 0] = g(h_0); out[:, t+1] = h_t

Sharding: 8 cores, core c -> (sample b = c//2, H-half hh = c%2, 256 channels).
Fully independent cores, no collectives; host assembles the output.

Device pipeline per 512-wide T-chunk, per 128-channel tile:
  - PE: fp8 e4m3 DoubleRow matmuls (0.5 cyc/row, 2 k-tiles per instruction).
    The f/i gates use plain x8@W8; the c gate adds two error-feedback terms
    (x8@(W-W8)8 + (x-x8)8@W8) which cut the end-to-end quantization error
    from ~2.0e-2 to ~0.9e-2 normwise.
  - Act: one Sigmoid instruction over the 3-gate PSUM tile [128, 3, TC].
  - DVE: fused custom op f = sa*~recip(sa+sb) (7 ALU stages, one pass);
    g = (pc + 0.5) max sg (scalar_tensor_tensor); tensor_tensor_scan.
  - Pool: v = (1-f)*g multiply; most of w = 1-f (ones - f subtract).
  - A fraction of w ops go to Act (Copy, scale=-1, bias=1) to balance.
Outputs stored f16 (DMA halved), upcast on host.
"""

from contextlib import ExitStack

import numpy as np
import ml_dtypes

import concourse.bacc as bacc
import concourse.tile as tile
import concourse.mybir as mybir
from concourse.bass_utils import run_bass_kernel_spmd

import concourse.dve_ops as _dve_ops
from concourse.dve_spec import (Spec as _Spec, Src0 as _S0, Src1 as _S1,
                                C1 as _C1, C2 as _C2,
                                AluOp as _AluOp, Bin as _Bin, lower as _lower)
from concourse.dve_uop import DveOpSpec as _DveOpSpec
from concourse.dve_table_gen import dve_ver_for as _dve_ver_for

# ---- fused custom DVE op: f = in0 * ~1/(in0+in1) ---------------------------
# x = in0+in1; nx = bitcast(~x) (exponent-flip reciprocal seed); u = x*nx
# lands in [-4.5,-4]; f = in0 * nx * (c1*u + c2), deg-1 minimax of 1/u on
# that interval (max rel err ~2.2e-3). 7 ALU stages -> single DVE pass.
GATE_FRAC_CONSTS = {"s1": -0.0546648, "imm2": -0.4681172}


def _register_gate_frac():
    name = "GATE_FRAC_ANT"
    if name in _dve_ops._SUB_OPCODE_FOR_NAME:
        return next(o for o in _dve_ops.OPS if o.name == name)
    _x = _S0 + _S1
    _nx = _Bin(_AluOp.BITWISE_NOT, _x, _x)
    _u = _x * _nx

    def _ref(in0, in1, c0, c1, c2):
        in0 = np.asarray(in0, np.float32)
        in1 = np.asarray(in1, np.float32)
        x = (in0 + in1).astype(np.float32)
        nx = (~x.view(np.int32)).view(np.float32)
        u = x * nx
        return (nx * (c1 * u + c2)) * in0

    spec = _Spec(body=(_Bin(_AluOp.MULTIPLY, _nx, (_C1 * _u + _C2))) * _S0,
                 reference=_ref)
    row = _dve_ops._CUSTOM_DVE_ROW_BASE + len(_dve_ops.OPS)
    assert row < 0x20
    ver = _dve_ver_for("TRN2")
    sha = _DveOpSpec(name=name, opcode=row, uops=_lower(spec, ver=ver),
                     rd1_en=True).sha(ver)
    op = _dve_ops.DveOp(name, spec, subdim=False, uops_sha={ver: sha})
    _dve_ops.OPS.append(op)
    _dve_ops.CUSTOM_DVE_SPECS[name] = spec
    _dve_ops._SUB_OPCODE_FOR_NAME[name] = row
    return op


_GATE_FRAC_OP = _register_gate_frac()

F32 = mybir.dt.float32
F16 = mybir.dt.float16
F8 = mybir.dt.float8e4
AF = mybir.ActivationFunctionType
OP = mybir.AluOpType
PM = mybir.MatmulPerfMode
E4 = ml_dtypes.float8_e4m3fn

B, T, D, H = 4, 8192, 512, 512
NCORES = 8
HS = H // 2          # 256 channels per core
TC = 512             # T chunk width
NCH = T // TC        # 16 chunks
NHT = HS // 128      # 2 h-tiles per core
NPAIR = 2            # DoubleRow k-tile pairs covering D=512

MM_MODE = "fp8dr"    # kept for test.py compatibility

# every W_ACT_EVERY-th (chunk,ht) unit computes w = 1-f on Act instead of
# Pool, balancing Act ~53us / Pool ~60us under the DVE ~63us bottleneck
W_ACT_EVERY = 1   # unused, kept for reference
# w-engine pattern: p=Pool, d=DVE, a=Act (balances the three engines)
W_PATTERN = "ppa"

_nc_cache = {}


def _build_nc(mm_mode=MM_MODE):
    nc = bacc.Bacc("TRN2", target_bir_lowering=False, debug=False,
                   num_devices=NCORES)
    # x8 (slots 0-3) and xlo8 (slots 4-7) k-tiles, chunk-major
    xq = nc.dram_tensor("xq", [NCH * 128, 8, TC], F8, kind="ExternalInput")
    w8 = nc.dram_tensor("w8", [128, NPAIR, 2, 3 * HS], F8,
                        kind="ExternalInput")
    wlo = nc.dram_tensor("wlo", [128, NPAIR, 2, 3 * HS], F8,
                          kind="ExternalInput")
    aux = nc.dram_tensor("aux", [128, NHT], F32, kind="ExternalInput")
    out = nc.dram_tensor("out", [128, NHT, T], F16, kind="ExternalOutput")

    with tile.TileContext(nc) as tc, ExitStack() as ctx:
        wpool = ctx.enter_context(tc.tile_pool(name="w", bufs=1))
        xpool = ctx.enter_context(tc.tile_pool(name="x", bufs=4))
        gpool = ctx.enter_context(tc.tile_pool(name="g", bufs=6))
        hpool = ctx.enter_context(tc.tile_pool(name="h", bufs=5))
        ppool = ctx.enter_context(tc.tile_pool(name="p", bufs=2, space="PSUM"))

        # weight/aux loads on the ACT HWDGE queue so the first x-chunk loads
        # (SP queue) run in parallel with them
        wt = wpool.tile([128, NPAIR, 2, 3 * HS], F8, tag="w8", name="wt")
        nc.scalar.dma_start(wt[:], w8[:])
        wlt = wpool.tile([128, NPAIR, 2, 3 * HS], F8, tag="wlo",
                         name="wlt")
        nc.scalar.dma_start(wlt[:], wlo[:])
        auxt = wpool.tile([128, NHT], F32, tag="aux", name="auxt")
        nc.scalar.dma_start(auxt[:], aux[:])
        ones = wpool.tile([128, TC], F16, tag="ones", name="ones")
        nc.vector.memset(ones[:], 1.0)

        # PE p-state warmup: the tensor engine needs ~3us of continuous work
        # to reach full clock. The first ~5us of the kernel are DMA-only, so
        # burn them on scratch matmuls (no data dependencies) and the first
        # real matmuls run at full speed.
        wu_s = wpool.tile([128, 2, 128], F8, tag="wu_s", name="wu_s")
        nc.vector.memset(wu_s[:], 0.0)
        wu_m = wpool.tile([128, 2, TC], F8, tag="wu_m", name="wu_m")
        nc.vector.memset(wu_m[:], 0.0)
        wu_p = ppool.tile([128, TC], F32, tag="Pc", name="wu_p", bufs=4)
        for _ in range(12):
            nc.tensor.matmul(wu_p[:], wu_s[:], wu_m[:], start=True,
                             stop=True, perf_mode=PM.DoubleRow)

        # chunk 0 split in half so the pipeline primes on half-size units
        chunks = [(0, TC // 2), (TC // 2, TC // 2)]
        chunks += [(k * TC, TC) for k in range(1, NCH)]

        NUNITS = len(chunks) * NHT
        carry = [None] * NHT
        it = 0
        # two-unit software pipeline: unit k's head [matmuls, sigmoid, g, f]
        # is emitted before unit k-2's tail [w, v, scan, out-dma]. The scan
        # of unit k-2 then never stalls the in-order DVE: its v input has
        # been computable since unit k-1's head ran.
        TAIL_DELAY = 2
        from collections import deque
        pendq = deque()  # (f, g, h_slice, ini, carry_ap, ht, ci, tw, unit#)
        out_dmas = {}    # ci -> (ht_tile, tsl, tw) emitted when ht=1 scanned

        def flush_pending(force=False):
            while pendq and (force or len(pendq) > TAIL_DELAY):
                f_, g_, h_, carry_ap, ht_, ci_, tw_, k_ = pendq.popleft()
                # resolve the scan initial lazily: the same-ht predecessor's
                # scan is emitted exactly TAIL_DELAY units before this flush
                ini_ = auxt[:, ht_:ht_ + 1] if ci_ == 0 else carry[ht_]
                # w = 1-f: Pool / DVE / Act per balance pattern; in the
                # drain (Act idle, Pool the laggard) w goes to Act and v to
                # DVE so the last scans issue as early as possible
                drain = k_ >= NUNITS - 3
                w_ = gpool.tile([128, TC], F16, tag="w", name="w_")[:, :tw_]
                wsel = "a" if drain else W_PATTERN[k_ % len(W_PATTERN)]
                if wsel == "a":
                    nc.scalar.activation(w_, f_, AF.Copy, bias=1.0,
                                         scale=-1.0)
                elif wsel == "d":
                    nc.vector.tensor_scalar(w_, f_, -1.0, 1.0,
                                            OP.mult, OP.add)
                else:
                    nc.gpsimd.tensor_tensor(w_, ones[:, :tw_], f_,
                                            op=OP.subtract)
                v_ = gpool.tile([128, TC], F16, tag="v", name="v_")[:, :tw_]
                if drain:
                    nc.vector.tensor_tensor(v_, w_, g_, op=OP.mult)
                else:
                    nc.gpsimd.tensor_tensor(v_, w_, g_, op=OP.mult)
                nc.vector.tensor_tensor_scan(h_, f_, v_, ini_, OP.mult,
                                             OP.add)
                carry[ht_] = carry_ap
                if ci_ == len(chunks) - 1:
                    htile_, tsl_, twc_ = out_dmas[ci_]
                    nc.sync.dma_start(out[:, ht_:ht_ + 1, tsl_],
                                      htile_[:, ht_:ht_ + 1, :twc_])
                elif ht_ == NHT - 1:
                    htile_, tsl_, twc_ = out_dmas.pop(ci_)
                    nc.sync.dma_start(out[:, :, tsl_], htile_[:, :, :twc_])

        for ci, (t0, tw) in enumerate(chunks):
            tsl = slice(t0, t0 + tw)
            xall = xpool.tile([128, 8, TC], F8, tag="xall", name="xall")
            crow = (t0 // TC) * 128
            csl = slice(t0 % TC, t0 % TC + tw)
            nc.sync.dma_start(xall[:, :, csl.start:csl.stop] if tw != TC
                              else xall[:],
                              xq[crow:crow + 128, :, csl])
            ht_tile = hpool.tile([128, NHT, TC], F16, tag="h", name="ht_tile")
            out_dmas[ci] = (ht_tile, tsl, tw)
            for ht in range(NHT):
                # split PSUM: Pc (1 bank, 4-deep rotation — its late reader
                # g never gates the PE) and Pab (2 banks, 2-deep — freed by
                # sigmoid-ab alone)
                Pc = ppool.tile([128, TC], F32, tag="Pc", name="Pc", bufs=4)
                Pab = ppool.tile([128, 2, TC], F32, tag="Pab", name="Pab",
                                 bufs=2)
                for gate in range(3):
                    c0 = gate * HS + ht * 128
                    dst = Pc[:, :tw] if gate == 2 else Pab[:, gate, :tw]
                    nterm = 3 if gate == 2 else 1
                    step = 0
                    for term in range(nterm):
                        for p in range(NPAIR):
                            if term == 1:       # x8 @ Wlo8
                                stat = wlt[:, p, :, c0:c0 + 128]
                                mov = xall[:, 2 * p:2 * p + 2, csl]
                            elif term == 2:     # xlo8 @ W8 (c gate only)
                                stat = wt[:, p, :, c0:c0 + 128]
                                mov = xall[:, 4 + 2 * p:4 + 2 * p + 2, csl]
                            else:               # x8 @ W8
                                stat = wt[:, p, :, c0:c0 + 128]
                                mov = xall[:, 2 * p:2 * p + 2, csl]
                            nc.tensor.matmul(
                                dst, stat, mov,
                                start=(step == 0),
                                stop=(step == 2 * nterm - 1),
                                perf_mode=PM.DoubleRow)
                            step += 1
                # sigmoid-c first (g on DVE consumes it), then sigmoid-ab
                sg = gpool.tile([128, TC], F16, tag="sg", name="sg")[:, :tw]
                nc.scalar.activation(sg, Pc[:, :tw], AF.Sigmoid)
                sab = gpool.tile([128, 2, TC], F16, tag="sab", name="sab")
                nc.scalar.activation(sab[:, :, :tw], Pab[:, :, :tw],
                                     AF.Sigmoid)
                # g = (pc + 0.5) max sg on DVE; last reader of Pc
                g = gpool.tile([128, TC], F16, tag="g", name="g")[:, :tw]
                nc.vector.scalar_tensor_tensor(g, Pc[:, :tw], 0.5, sg,
                                               OP.add, OP.max)
                # f = sa/(sa+sb) fused on DVE
                f = gpool.tile([128, TC], F16, tag="f", name="f")[:, :tw]
                nc.vector._custom_dve(_GATE_FRAC_OP, out=f,
                                      in0=sab[:, 0, :tw], in1=sab[:, 1, :tw],
                                      s0=0.0, s1=GATE_FRAC_CONSTS["s1"],
                                      imm2=GATE_FRAC_CONSTS["imm2"])
                # emit the PREVIOUS unit's tail (w, v, scan, out-dma) now
                # that this unit's head is queued ahead of it
                flush_pending()
                h = ht_tile[:, ht, :tw]
                carry_ap = ht_tile[:, ht, tw - 1:tw]
                pendq.append((f, g, h, carry_ap, ht, ci, tw, it))
                it += 1
        flush_pending(force=True)
    nc.compile()
    return nc


def _get_nc(mm_mode=MM_MODE):
    if mm_mode not in _nc_cache:
        _nc_cache[mm_mode] = _build_nc(mm_mode)
    return _nc_cache[mm_mode]


def _g_host(x):
    # exp(log_g(x)) of the reference, computed directly in fp32
    return np.where(x >= 0, x + 0.5, 1.0 / (1.0 + np.exp(-np.minimum(x, 0))))


def _prep_x(xb):
    """xb [T, D] f32 -> [NCH*128, 8, TC] e4m3 (x8 + xlo8 k-tiles)."""
    xt = np.ascontiguousarray(xb.reshape(NCH, TC, D).transpose(0, 2, 1))
    x8 = xt.astype(E4)                                   # [NCH, D, TC]
    xlo8 = (xt - x8.astype(np.float32)).astype(E4)
    # [NCH, 8, 128, TC] slot-major -> [NCH, 128, 8, TC]
    both = np.concatenate([x8.reshape(NCH, 4, 128, TC),
                           xlo8.reshape(NCH, 4, 128, TC)], axis=1)
    return np.ascontiguousarray(both.transpose(0, 2, 1, 3)).reshape(
        NCH * 128, 8, TC)


def _prep_w(wcat):
    """[D, C] f32 -> ([128, NPAIR, 2, C] e4m3 base, same-shape lo residual
    of the last HS columns)."""
    w8 = wcat.astype(E4)
    dev = np.ascontiguousarray(
        w8.reshape(NPAIR, 2, 128, wcat.shape[1]).transpose(2, 0, 1, 3))
    return dev


def _run(inputs, mm_mode=MM_MODE, trace=False):
    x = np.asarray(inputs["x"], np.float32)
    h_0 = np.asarray(inputs["h_0"], np.float32)
    W_f = np.asarray(inputs["W_f"], np.float32)
    b_f = np.asarray(inputs["b_f"], np.float32)
    W_i = np.asarray(inputs["W_i"], np.float32)
    b_i = np.asarray(inputs["b_i"], np.float32)
    W_h = np.asarray(inputs["W_h"], np.float32)
    b_h = np.asarray(inputs["b_h"], np.float32)
    assert (b_f == 0).all() and (b_i == 0).all() and (b_h == 0).all(), \
        "device program folds zero biases"

    g0 = _g_host(h_0[:, 0, :])  # [B, H]
    xqs = [_prep_x(x[b]) for b in range(B)]

    in_maps = []
    for c in range(NCORES):
        b, hh = divmod(c, 2)
        hs = slice(hh * HS, (hh + 1) * HS)
        wcat = np.concatenate([W_f[:, hs], W_i[:, hs], W_h[:, hs]], axis=1)
        w8dev = _prep_w(wcat)
        wlo_cat = wcat - wcat.astype(E4).astype(np.float32)
        wlodev = np.ascontiguousarray(
            wlo_cat.astype(E4).reshape(NPAIR, 2, 128,
                                       3 * HS).transpose(2, 0, 1, 3))
        auxa = np.ascontiguousarray(
            g0[b, hs].reshape(NHT, 128).T.astype(np.float32))
        in_maps.append({"xq": xqs[b], "w8": w8dev, "wlo": wlodev,
                        "aux": auxa})

    nc = _get_nc(mm_mode)
    res = run_bass_kernel_spmd(nc, in_maps, core_ids=list(range(NCORES)),
                               trace=trace)

    out = np.empty((B, T + 1, H), np.float32)
    out[:, 0, :] = g0
    for c in range(NCORES):
        b, hh = divmod(c, 2)
        hs = slice(hh * HS, (hh + 1) * HS)
        blk = res.results[c]["out"].astype(np.float32)   # [128, NHT, T]
        out[b, 1:, hs] = blk.transpose(2, 1, 0).reshape(T, HS)
    return out, res


def kernel(**inputs):
    out, _ = _run(inputs)
    return out


# revision 24
# speedup vs baseline: 1.0175x; 1.0097x over previous
"""MinLSTM Trainium2 kernel (fp8 DoubleRow edition).

Full-input contract: kernel(**inputs) takes the complete (unsharded) numpy
inputs of the reference model and returns the full [B, T+1, H] float32 output.

Math (per batch b, channel h — identical to the reference's log-space scan,
computed in linear space; every quantity is positive so the linear recurrence
is numerically stable):
    a = x @ W_f ;  b = x @ W_i ;  c = x @ W_h          (zero biases folded)
    f = sigmoid(a) / (sigmoid(a) + sigmoid(b))          # forget gate
    i = 1 - f                                           # input gate
    g = max(c + 0.5, sigmoid(c))                        # = exp(log_g(c))
    h_t = f_t * h_{t-1} + i_t * g_t,   h_{-1} = g(h_0)
    out[:, 0] = g(h_0); out[:, t+1] = h_t

Sharding: 8 cores, core c -> (sample b = c//2, H-half hh = c%2, 256 channels).
Fully independent cores, no collectives; host assembles the output.

Device pipeline per 512-wide T-chunk, per 128-channel tile:
  - PE: fp8 e4m3 DoubleRow matmuls (0.5 cyc/row, 2 k-tiles per instruction).
    The f/i gates use plain x8@W8; the c gate adds two error-feedback terms
    (x8@(W-W8)8 + (x-x8)8@W8) which cut the end-to-end quantization error
    from ~2.0e-2 to ~0.9e-2 normwise.
  - Act: one Sigmoid instruction over the 3-gate PSUM tile [128, 3, TC].
  - DVE: fused custom op f = sa*~recip(sa+sb) (7 ALU stages, one pass);
    g = (pc + 0.5) max sg (scalar_tensor_tensor); tensor_tensor_scan.
  - Pool: v = (1-f)*g multiply; most of w = 1-f (ones - f subtract).
  - A fraction of w ops go to Act (Copy, scale=-1, bias=1) to balance.
Outputs stored f16 (DMA halved), upcast on host.
"""

from contextlib import ExitStack

import numpy as np
import ml_dtypes

import concourse.bacc as bacc
import concourse.tile as tile
import concourse.mybir as mybir
from concourse.bass_utils import run_bass_kernel_spmd

import concourse.dve_ops as _dve_ops
from concourse.dve_spec import (Spec as _Spec, Src0 as _S0, Src1 as _S1,
                                C1 as _C1, C2 as _C2,
                                AluOp as _AluOp, Bin as _Bin, lower as _lower)
from concourse.dve_uop import DveOpSpec as _DveOpSpec
from concourse.dve_table_gen import dve_ver_for as _dve_ver_for

# ---- fused custom DVE op: f = in0 * ~1/(in0+in1) ---------------------------
# x = in0+in1; nx = bitcast(~x) (exponent-flip reciprocal seed); u = x*nx
# lands in [-4.5,-4]; f = in0 * nx * (c1*u + c2), deg-1 minimax of 1/u on
# that interval (max rel err ~2.2e-3). 7 ALU stages -> single DVE pass.
GATE_FRAC_CONSTS = {"s1": -0.0546648, "imm2": -0.4681172}


def _register_gate_frac():
    name = "GATE_FRAC_ANT"
    if name in _dve_ops._SUB_OPCODE_FOR_NAME:
        return next(o for o in _dve_ops.OPS if o.name == name)
    _x = _S0 + _S1
    _nx = _Bin(_AluOp.BITWISE_NOT, _x, _x)
    _u = _x * _nx

    def _ref(in0, in1, c0, c1, c2):
        in0 = np.asarray(in0, np.float32)
        in1 = np.asarray(in1, np.float32)
        x = (in0 + in1).astype(np.float32)
        nx = (~x.view(np.int32)).view(np.float32)
        u = x * nx
        return (nx * (c1 * u + c2)) * in0

    spec = _Spec(body=(_Bin(_AluOp.MULTIPLY, _nx, (_C1 * _u + _C2))) * _S0,
                 reference=_ref)
    row = _dve_ops._CUSTOM_DVE_ROW_BASE + len(_dve_ops.OPS)
    assert row < 0x20
    ver = _dve_ver_for("TRN2")
    sha = _DveOpSpec(name=name, opcode=row, uops=_lower(spec, ver=ver),
                     rd1_en=True).sha(ver)
    op = _dve_ops.DveOp(name, spec, subdim=False, uops_sha={ver: sha})
    _dve_ops.OPS.append(op)
    _dve_ops.CUSTOM_DVE_SPECS[name] = spec
    _dve_ops._SUB_OPCODE_FOR_NAME[name] = row
    return op


_GATE_FRAC_OP = _register_gate_frac()

F32 = mybir.dt.float32
F16 = mybir.dt.float16
F8 = mybir.dt.float8e4
AF = mybir.ActivationFunctionType
OP = mybir.AluOpType
PM = mybir.MatmulPerfMode
E4 = ml_dtypes.float8_e4m3fn

B, T, D, H = 4, 8192, 512, 512
NCORES = 8
HS = H // 2          # 256 channels per core
TC = 512             # T chunk width
NCH = T // TC        # 16 chunks
NHT = HS // 128      # 2 h-tiles per core
NPAIR = 2            # DoubleRow k-tile pairs covering D=512

MM_MODE = "fp8dr"    # kept for test.py compatibility

# every W_ACT_EVERY-th (chunk,ht) unit computes w = 1-f on Act instead of
# Pool, balancing Act ~53us / Pool ~60us under the DVE ~63us bottleneck
W_ACT_EVERY = 1   # unused, kept for reference
# w-engine pattern: p=Pool, d=DVE, a=Act (balances the three engines)
W_PATTERN = "ppa"

_nc_cache = {}


def _build_nc(mm_mode=MM_MODE):
    nc = bacc.Bacc("TRN2", target_bir_lowering=False, debug=False,
                   num_devices=NCORES)
    # x8 (slots 0-3) and xlo8 (slots 4-7) k-tiles, chunk-major
    xq = nc.dram_tensor("xq", [NCH * 128, 8, TC], F8, kind="ExternalInput")
    w8 = nc.dram_tensor("w8", [128, NPAIR, 2, 3 * HS], F8,
                        kind="ExternalInput")
    wlo = nc.dram_tensor("wlo", [128, NPAIR, 2, HS], F8,
                         kind="ExternalInput")
    aux = nc.dram_tensor("aux", [128, NHT], F32, kind="ExternalInput")
    out = nc.dram_tensor("out", [128, NHT, T], F16, kind="ExternalOutput")

    with tile.TileContext(nc) as tc, ExitStack() as ctx:
        wpool = ctx.enter_context(tc.tile_pool(name="w", bufs=1))
        xpool = ctx.enter_context(tc.tile_pool(name="x", bufs=4))
        gpool = ctx.enter_context(tc.tile_pool(name="g", bufs=6))
        hpool = ctx.enter_context(tc.tile_pool(name="h", bufs=5))
        ppool = ctx.enter_context(tc.tile_pool(name="p", bufs=2, space="PSUM"))

        # weight/aux loads on the ACT HWDGE queue so the first x-chunk loads
        # (SP queue) run in parallel with them
        wt = wpool.tile([128, NPAIR, 2, 3 * HS], F8, tag="w8", name="wt")
        nc.scalar.dma_start(wt[:], w8[:])
        wlt = wpool.tile([128, NPAIR, 2, HS], F8, tag="wlo", name="wlt")
        nc.scalar.dma_start(wlt[:], wlo[:])
        auxt = wpool.tile([128, NHT], F32, tag="aux", name="auxt")
        nc.scalar.dma_start(auxt[:], aux[:])
        ones = wpool.tile([128, TC], F16, tag="ones", name="ones")
        nc.vector.memset(ones[:], 1.0)

        # PE p-state warmup: the tensor engine needs ~3us of continuous work
        # to reach full clock. The first ~5us of the kernel are DMA-only, so
        # burn them on scratch matmuls (no data dependencies) and the first
        # real matmuls run at full speed.
        wu_s = wpool.tile([128, 2, 128], F8, tag="wu_s", name="wu_s")
        nc.vector.memset(wu_s[:], 0.0)
        wu_m = wpool.tile([128, 2, TC], F8, tag="wu_m", name="wu_m")
        nc.vector.memset(wu_m[:], 0.0)
        wu_p = ppool.tile([128, TC], F32, tag="Pc", name="wu_p", bufs=4)
        for _ in range(12):
            nc.tensor.matmul(wu_p[:], wu_s[:], wu_m[:], start=True,
                             stop=True, perf_mode=PM.DoubleRow)

        # chunk 0 split in half so the pipeline primes on half-size units
        chunks = [(0, TC // 2), (TC // 2, TC // 2)]
        chunks += [(k * TC, TC) for k in range(1, NCH)]

        NUNITS = len(chunks) * NHT
        carry = [None] * NHT
        it = 0
        # two-unit software pipeline: unit k's head [matmuls, sigmoid, g, f]
        # is emitted before unit k-2's tail [w, v, scan, out-dma]. The scan
        # of unit k-2 then never stalls the in-order DVE: its v input has
        # been computable since unit k-1's head ran.
        TAIL_DELAY = 2
        from collections import deque
        pendq = deque()  # (f, g, h_slice, ini, carry_ap, ht, ci, tw, unit#)
        out_dmas = {}    # ci -> (ht_tile, tsl, tw) emitted when ht=1 scanned

        def flush_pending(force=False):
            while pendq and (force or len(pendq) > TAIL_DELAY):
                f_, g_, h_, carry_ap, ht_, ci_, tw_, k_ = pendq.popleft()
                # resolve the scan initial lazily: the same-ht predecessor's
                # scan is emitted exactly TAIL_DELAY units before this flush
                ini_ = auxt[:, ht_:ht_ + 1] if ci_ == 0 else carry[ht_]
                # w = 1-f: Pool / DVE / Act per balance pattern; in the
                # drain (Act idle, Pool the laggard) w goes to Act and v to
                # DVE so the last scans issue as early as possible
                drain = k_ >= NUNITS - 3
                w_ = gpool.tile([128, TC], F16, tag="w", name="w_")[:, :tw_]
                wsel = "a" if drain else W_PATTERN[k_ % len(W_PATTERN)]
                if wsel == "a":
                    nc.scalar.activation(w_, f_, AF.Copy, bias=1.0,
                                         scale=-1.0)
                elif wsel == "d":
                    nc.vector.tensor_scalar(w_, f_, -1.0, 1.0,
                                            OP.mult, OP.add)
                else:
                    nc.gpsimd.tensor_tensor(w_, ones[:, :tw_], f_,
                                            op=OP.subtract)
                v_ = gpool.tile([128, TC], F16, tag="v", name="v_")[:, :tw_]
                if drain:
                    nc.vector.tensor_tensor(v_, w_, g_, op=OP.mult)
                else:
                    nc.gpsimd.tensor_tensor(v_, w_, g_, op=OP.mult)
                nc.vector.tensor_tensor_scan(h_, f_, v_, ini_, OP.mult,
                                             OP.add)
                carry[ht_] = carry_ap
                if ci_ == len(chunks) - 1:
                    htile_, tsl_, twc_ = out_dmas[ci_]
                    nc.sync.dma_start(out[:, ht_:ht_ + 1, tsl_],
                                      htile_[:, ht_:ht_ + 1, :twc_])
                elif ht_ == NHT - 1:
                    htile_, tsl_, twc_ = out_dmas.pop(ci_)
                    nc.sync.dma_start(out[:, :, tsl_], htile_[:, :, :twc_])

        for ci, (t0, tw) in enumerate(chunks):
            tsl = slice(t0, t0 + tw)
            xall = xpool.tile([128, 8, TC], F8, tag="xall", name="xall")
            crow = (t0 // TC) * 128
            csl = slice(t0 % TC, t0 % TC + tw)
            nc.sync.dma_start(xall[:, :, csl.start:csl.stop] if tw != TC
                              else xall[:],
                              xq[crow:crow + 128, :, csl])
            ht_tile = hpool.tile([128, NHT, TC], F16, tag="h", name="ht_tile")
            out_dmas[ci] = (ht_tile, tsl, tw)
            for ht in range(NHT):
                # split PSUM: Pc (1 bank, 4-deep rotation — its late reader
                # g never gates the PE) and Pab (2 banks, 2-deep — freed by
                # sigmoid-ab alone)
                Pc = ppool.tile([128, TC], F32, tag="Pc", name="Pc", bufs=4)
                Pab = ppool.tile([128, 2, TC], F32, tag="Pab", name="Pab",
                                 bufs=2)
                for gate in range(3):
                    c0 = gate * HS + ht * 128
                    dst = Pc[:, :tw] if gate == 2 else Pab[:, gate, :tw]
                    nterm = 3 if gate == 2 else 1
                    step = 0
                    for term in range(nterm):
                        for p in range(NPAIR):
                            if term == 1:       # x8 @ Wlo8 (c gate only)
                                stat = wlt[:, p, :, ht * 128:ht * 128 + 128]
                                mov = xall[:, 2 * p:2 * p + 2, csl]
                            elif term == 2:     # xlo8 @ W8 (c gate only)
                                stat = wt[:, p, :, c0:c0 + 128]
                                mov = xall[:, 4 + 2 * p:4 + 2 * p + 2, csl]
                            else:               # x8 @ W8
                                stat = wt[:, p, :, c0:c0 + 128]
                                mov = xall[:, 2 * p:2 * p + 2, csl]
                            nc.tensor.matmul(
                                dst, stat, mov,
                                start=(step == 0),
                                stop=(step == 2 * nterm - 1),
                                perf_mode=PM.DoubleRow)
                            step += 1
                # sigmoid-c first (g on DVE consumes it), then sigmoid-ab
                sg = gpool.tile([128, TC], F16, tag="sg", name="sg")[:, :tw]
                nc.scalar.activation(sg, Pc[:, :tw], AF.Sigmoid)
                sab = gpool.tile([128, 2, TC], F16, tag="sab", name="sab")
                nc.scalar.activation(sab[:, :, :tw], Pab[:, :, :tw],
                                     AF.Sigmoid)
                # g = (pc + 0.5) max sg on DVE; last reader of Pc
                g = gpool.tile([128, TC], F16, tag="g", name="g")[:, :tw]
                nc.vector.scalar_tensor_tensor(g, Pc[:, :tw], 0.5, sg,
                                               OP.add, OP.max)
                # f = sa/(sa+sb) fused on DVE
                f = gpool.tile([128, TC], F16, tag="f", name="f")[:, :tw]
                nc.vector._custom_dve(_GATE_FRAC_OP, out=f,
                                      in0=sab[:, 0, :tw], in1=sab[:, 1, :tw],
                                      s0=0.0, s1=GATE_FRAC_CONSTS["s1"],
                                      imm2=GATE_FRAC_CONSTS["imm2"])
                # emit the PREVIOUS unit's tail (w, v, scan, out-dma) now
                # that this unit's head is queued ahead of it
                flush_pending()
                h = ht_tile[:, ht, :tw]
                carry_ap = ht_tile[:, ht, tw - 1:tw]
                pendq.append((f, g, h, carry_ap, ht, ci, tw, it))
                it += 1
        flush_pending(force=True)
    nc.compile()
    return nc


def _get_nc(mm_mode=MM_MODE):
    if mm_mode not in _nc_cache:
        _nc_cache[mm_mode] = _build_nc(mm_mode)
    return _nc_cache[mm_mode]


def _g_host(x):
    # exp(log_g(x)) of the reference, computed directly in fp32
    return np.where(x >= 0, x + 0.5, 1.0 / (1.0 + np.exp(-np.minimum(x, 0))))


def _prep_x(xb):
    """xb [T, D] f32 -> [NCH*128, 8, TC] e4m3 (x8 + xlo8 k-tiles)."""
    xt = np.ascontiguousarray(xb.reshape(NCH, TC, D).transpose(0, 2, 1))
    x8 = xt.astype(E4)                                   # [NCH, D, TC]
    xlo8 = (xt - x8.astype(np.float32)).astype(E4)
    # [NCH, 8, 128, TC] slot-major -> [NCH, 128, 8, TC]
    both = np.concatenate([x8.reshape(NCH, 4, 128, TC),
                           xlo8.reshape(NCH, 4, 128, TC)], axis=1)
    return np.ascontiguousarray(both.transpose(0, 2, 1, 3)).reshape(
        NCH * 128, 8, TC)


def _prep_w(wcat):
    """[D, C] f32 -> ([128, NPAIR, 2, C] e4m3 base, same-shape lo residual
    of the last HS columns)."""
    w8 = wcat.astype(E4)
    dev = np.ascontiguousarray(
        w8.reshape(NPAIR, 2, 128, wcat.shape[1]).transpose(2, 0, 1, 3))
    return dev


def _run(inputs, mm_mode=MM_MODE, trace=False):
    x = np.asarray(inputs["x"], np.float32)
    h_0 = np.asarray(inputs["h_0"], np.float32)
    W_f = np.asarray(inputs["W_f"], np.float32)
    b_f = np.asarray(inputs["b_f"], np.float32)
    W_i = np.asarray(inputs["W_i"], np.float32)
    b_i = np.asarray(inputs["b_i"], np.float32)
    W_h = np.asarray(inputs["W_h"], np.float32)
    b_h = np.asarray(inputs["b_h"], np.float32)
    assert (b_f == 0).all() and (b_i == 0).all() and (b_h == 0).all(), \
        "device program folds zero biases"

    g0 = _g_host(h_0[:, 0, :])  # [B, H]
    xqs = [_prep_x(x[b]) for b in range(B)]

    in_maps = []
    for c in range(NCORES):
        b, hh = divmod(c, 2)
        hs = slice(hh * HS, (hh + 1) * HS)
        wcat = np.concatenate([W_f[:, hs], W_i[:, hs], W_h[:, hs]], axis=1)
        w8dev = _prep_w(wcat)
        wh = W_h[:, hs]
        whlo = (wh - wh.astype(E4).astype(np.float32))
        wlodev = np.ascontiguousarray(
            whlo.astype(E4).reshape(NPAIR, 2, 128, HS).transpose(2, 0, 1, 3))
        auxa = np.ascontiguousarray(
            g0[b, hs].reshape(NHT, 128).T.astype(np.float32))
        in_maps.append({"xq": xqs[b], "w8": w8dev, "wlo": wlodev,
                        "aux": auxa})

    nc = _get_nc(mm_mode)
    res = run_bass_kernel_spmd(nc, in_maps, core_ids=list(range(NCORES)),
                               trace=trace)

    out = np.empty((B, T + 1, H), np.float32)
    out[:, 0, :] = g0
    for c in range(NCORES):
        b, hh = divmod(c, 2)
        hs = slice(hh * HS, (hh + 1) * HS)
        blk = res.results[c]["out"].astype(np.float32)   # [128, NHT, T]
        out[b, 1:, hs] = blk.transpose(2, 1, 0).reshape(T, HS)
    return out, res


def kernel(**inputs):
    out, _ = _run(inputs)
    return out


# revision 25
# speedup vs baseline: 1.0300x; 1.0123x over previous
"""MinLSTM Trainium2 kernel (fp8 DoubleRow edition).

Full-input contract: kernel(**inputs) takes the complete (unsharded) numpy
inputs of the reference model and returns the full [B, T+1, H] float32 output.

Math (per batch b, channel h — identical to the reference's log-space scan,
computed in linear space; every quantity is positive so the linear recurrence
is numerically stable):
    a = x @ W_f ;  b = x @ W_i ;  c = x @ W_h          (zero biases folded)
    f = sigmoid(a) / (sigmoid(a) + sigmoid(b))          # forget gate
    i = 1 - f                                           # input gate
    g = max(c + 0.5, sigmoid(c))                        # = exp(log_g(c))
    h_t = f_t * h_{t-1} + i_t * g_t,   h_{-1} = g(h_0)
    out[:, 0] = g(h_0); out[:, t+1] = h_t

Sharding: 8 cores, core c -> (sample b = c//2, H-half hh = c%2, 256 channels).
Fully independent cores, no collectives; host assembles the output.

Device pipeline per 512-wide T-chunk, per 128-channel tile:
  - PE: fp8 e4m3 DoubleRow matmuls (0.5 cyc/row, 2 k-tiles per instruction).
    The f/i gates use plain x8@W8; the c gate adds two error-feedback terms
    (x8@(W-W8)8 + (x-x8)8@W8) which cut the end-to-end quantization error
    from ~2.0e-2 to ~0.9e-2 normwise.
  - Act: one Sigmoid instruction over the 3-gate PSUM tile [128, 3, TC].
  - DVE: fused custom op f = sa*~recip(sa+sb) (7 ALU stages, one pass);
    g = (pc + 0.5) max sg (scalar_tensor_tensor); tensor_tensor_scan.
  - Pool: v = (1-f)*g multiply; most of w = 1-f (ones - f subtract).
  - A fraction of w ops go to Act (Copy, scale=-1, bias=1) to balance.
Outputs stored f16 (DMA halved), upcast on host.
"""

from contextlib import ExitStack

import numpy as np
import ml_dtypes

import concourse.bacc as bacc
import concourse.tile as tile
import concourse.mybir as mybir
from concourse.bass_utils import run_bass_kernel_spmd

import concourse.dve_ops as _dve_ops
from concourse.dve_spec import (Spec as _Spec, Src0 as _S0, Src1 as _S1,
                                C1 as _C1, C2 as _C2,
                                AluOp as _AluOp, Bin as _Bin, lower as _lower)
from concourse.dve_uop import DveOpSpec as _DveOpSpec
from concourse.dve_table_gen import dve_ver_for as _dve_ver_for

# ---- fused custom DVE op: f = in0 * ~1/(in0+in1) ---------------------------
# x = in0+in1; nx = bitcast(~x) (exponent-flip reciprocal seed); u = x*nx
# lands in [-4.5,-4]; f = in0 * nx * (c1*u + c2), deg-1 minimax of 1/u on
# that interval (max rel err ~2.2e-3). 7 ALU stages -> single DVE pass.
GATE_FRAC_CONSTS = {"s1": -0.0546648, "imm2": -0.4681172}


def _register_gate_frac():
    name = "GATE_FRAC_ANT"
    if name in _dve_ops._SUB_OPCODE_FOR_NAME:
        return next(o for o in _dve_ops.OPS if o.name == name)
    _x = _S0 + _S1
    _nx = _Bin(_AluOp.BITWISE_NOT, _x, _x)
    _u = _x * _nx

    def _ref(in0, in1, c0, c1, c2):
        in0 = np.asarray(in0, np.float32)
        in1 = np.asarray(in1, np.float32)
        x = (in0 + in1).astype(np.float32)
        nx = (~x.view(np.int32)).view(np.float32)
        u = x * nx
        return (nx * (c1 * u + c2)) * in0

    spec = _Spec(body=(_Bin(_AluOp.MULTIPLY, _nx, (_C1 * _u + _C2))) * _S0,
                 reference=_ref)
    row = _dve_ops._CUSTOM_DVE_ROW_BASE + len(_dve_ops.OPS)
    assert row < 0x20
    ver = _dve_ver_for("TRN2")
    sha = _DveOpSpec(name=name, opcode=row, uops=_lower(spec, ver=ver),
                     rd1_en=True).sha(ver)
    op = _dve_ops.DveOp(name, spec, subdim=False, uops_sha={ver: sha})
    _dve_ops.OPS.append(op)
    _dve_ops.CUSTOM_DVE_SPECS[name] = spec
    _dve_ops._SUB_OPCODE_FOR_NAME[name] = row
    return op


_GATE_FRAC_OP = _register_gate_frac()

F32 = mybir.dt.float32
F16 = mybir.dt.float16
F8 = mybir.dt.float8e4
AF = mybir.ActivationFunctionType
OP = mybir.AluOpType
PM = mybir.MatmulPerfMode
E4 = ml_dtypes.float8_e4m3fn

B, T, D, H = 4, 8192, 512, 512
NCORES = 8
HS = H // 2          # 256 channels per core
TC = 512             # T chunk width
NCH = T // TC        # 16 chunks
NHT = HS // 128      # 2 h-tiles per core
NPAIR = 2            # DoubleRow k-tile pairs covering D=512

MM_MODE = "fp8dr"    # kept for test.py compatibility

# every W_ACT_EVERY-th (chunk,ht) unit computes w = 1-f on Act instead of
# Pool, balancing Act ~53us / Pool ~60us under the DVE ~63us bottleneck
W_ACT_EVERY = 1   # unused, kept for reference
# w-engine pattern: p=Pool, d=DVE, a=Act (balances the three engines)
W_PATTERN = "ppa"

_nc_cache = {}


def _build_nc(mm_mode=MM_MODE):
    nc = bacc.Bacc("TRN2", target_bir_lowering=False, debug=False,
                   num_devices=NCORES)
    # x8 (slots 0-3) and xlo8 (slots 4-7) k-tiles, chunk-major
    xq = nc.dram_tensor("xq", [NCH * 128, 8, TC], F8, kind="ExternalInput")
    w8 = nc.dram_tensor("w8", [128, NPAIR, 2, 3 * HS], F8,
                        kind="ExternalInput")
    wlo = nc.dram_tensor("wlo", [128, NPAIR, 2, HS], F8,
                         kind="ExternalInput")
    aux = nc.dram_tensor("aux", [128, NHT], F32, kind="ExternalInput")
    out = nc.dram_tensor("out", [128, NHT, T], F16, kind="ExternalOutput")

    with tile.TileContext(nc) as tc, ExitStack() as ctx:
        wpool = ctx.enter_context(tc.tile_pool(name="w", bufs=1))
        xpool = ctx.enter_context(tc.tile_pool(name="x", bufs=4))
        gpool = ctx.enter_context(tc.tile_pool(name="g", bufs=6))
        hpool = ctx.enter_context(tc.tile_pool(name="h", bufs=5))
        ppool = ctx.enter_context(tc.tile_pool(name="p", bufs=2, space="PSUM"))

        # weight/aux loads on the ACT HWDGE queue so the first x-chunk loads
        # (SP queue) run in parallel with them
        wt = wpool.tile([128, NPAIR, 2, 3 * HS], F8, tag="w8", name="wt")
        nc.scalar.dma_start(wt[:], w8[:])
        wlt = wpool.tile([128, NPAIR, 2, HS], F8, tag="wlo", name="wlt")
        nc.scalar.dma_start(wlt[:], wlo[:])
        auxt = wpool.tile([128, NHT], F32, tag="aux", name="auxt")
        nc.scalar.dma_start(auxt[:], aux[:])
        ones = wpool.tile([128, TC], F16, tag="ones", name="ones")
        nc.vector.memset(ones[:], 1.0)

        # PE p-state warmup: the tensor engine needs ~3us of continuous work
        # to reach full clock. The first ~5us of the kernel are DMA-only, so
        # burn them on scratch matmuls (no data dependencies) and the first
        # real matmuls run at full speed.
        wu_s = wpool.tile([128, 2, 128], F8, tag="wu_s", name="wu_s")
        nc.vector.memset(wu_s[:], 0.0)
        wu_m = wpool.tile([128, 2, TC], F8, tag="wu_m", name="wu_m")
        nc.vector.memset(wu_m[:], 0.0)
        wu_p = ppool.tile([128, TC], F32, tag="Pc", name="wu_p", bufs=4)
        for _ in range(12):
            nc.tensor.matmul(wu_p[:], wu_s[:], wu_m[:], start=True,
                             stop=True, perf_mode=PM.DoubleRow)

        # chunk 0 split in half so the pipeline primes on half-size units
        chunks = [(0, TC // 2), (TC // 2, TC // 2)]
        chunks += [(k * TC, TC) for k in range(1, NCH)]

        NUNITS = len(chunks) * NHT
        carry = [None] * NHT
        it = 0
        # two-unit software pipeline: unit k's head [matmuls, sigmoid, g, f]
        # is emitted before unit k-2's tail [w, v, scan, out-dma]. The scan
        # of unit k-2 then never stalls the in-order DVE: its v input has
        # been computable since unit k-1's head ran.
        TAIL_DELAY = 2
        from collections import deque
        pendq = deque()  # (f, g, h_slice, ini, carry_ap, ht, ci, tw, unit#)
        out_dmas = {}    # ci -> (ht_tile, tsl, tw) emitted when ht=1 scanned

        def flush_pending(force=False):
            while pendq and (force or len(pendq) > TAIL_DELAY):
                f_, g_, h_, carry_ap, ht_, ci_, tw_, k_ = pendq.popleft()
                # resolve the scan initial lazily: the same-ht predecessor's
                # scan is emitted exactly TAIL_DELAY units before this flush
                ini_ = auxt[:, ht_:ht_ + 1] if ci_ == 0 else carry[ht_]
                # w = 1-f: Pool / DVE / Act per balance pattern; in the
                # drain (Act idle, Pool the laggard) w goes to Act and v to
                # DVE so the last scans issue as early as possible
                drain = k_ >= NUNITS - 3
                w_ = gpool.tile([128, TC], F16, tag="w", name="w_")[:, :tw_]
                wsel = "a" if drain else W_PATTERN[k_ % len(W_PATTERN)]
                if wsel == "a":
                    nc.scalar.activation(w_, f_, AF.Copy, bias=1.0,
                                         scale=-1.0)
                elif wsel == "d":
                    nc.vector.tensor_scalar(w_, f_, -1.0, 1.0,
                                            OP.mult, OP.add)
                else:
                    nc.gpsimd.tensor_tensor(w_, ones[:, :tw_], f_,
                                            op=OP.subtract)
                v_ = gpool.tile([128, TC], F16, tag="v", name="v_")[:, :tw_]
                if drain:
                    nc.vector.tensor_tensor(v_, w_, g_, op=OP.mult)
                else:
                    nc.gpsimd.tensor_tensor(v_, w_, g_, op=OP.mult)
                nc.vector.tensor_tensor_scan(h_, f_, v_, ini_, OP.mult,
                                             OP.add)
                carry[ht_] = carry_ap
                if ci_ == len(chunks) - 1:
                    htile_, tsl_, twc_ = out_dmas[ci_]
                    nc.sync.dma_start(out[:, ht_:ht_ + 1, tsl_],
                                      htile_[:, ht_:ht_ + 1, :twc_])
                elif ht_ == NHT - 1:
                    htile_, tsl_, twc_ = out_dmas.pop(ci_)
                    nc.sync.dma_start(out[:, :, tsl_], htile_[:, :, :twc_])

        for ci, (t0, tw) in enumerate(chunks):
            tsl = slice(t0, t0 + tw)
            xall = xpool.tile([128, 8, TC], F8, tag="xall", name="xall")
            crow = (t0 // TC) * 128
            csl = slice(t0 % TC, t0 % TC + tw)
            if ci == 0:
                # first chunk: land the x8 k-tiles (slots 0-3) before the
                # xlo slots so the f/i matmuls can start sooner
                nc.sync.dma_start(xall[:, :4, csl.start:csl.stop],
                                  xq[crow:crow + 128, :4, csl])
                nc.sync.dma_start(xall[:, 4:, csl.start:csl.stop],
                                  xq[crow:crow + 128, 4:, csl])
            else:
                nc.sync.dma_start(xall[:, :, csl.start:csl.stop] if tw != TC
                                  else xall[:],
                                  xq[crow:crow + 128, :, csl])
            ht_tile = hpool.tile([128, NHT, TC], F16, tag="h", name="ht_tile")
            out_dmas[ci] = (ht_tile, tsl, tw)
            for ht in range(NHT):
                # split PSUM: Pc (1 bank, 4-deep rotation — its late reader
                # g never gates the PE) and Pab (2 banks, 2-deep — freed by
                # sigmoid-ab alone)
                Pc = ppool.tile([128, TC], F32, tag="Pc", name="Pc", bufs=4)
                Pab = ppool.tile([128, 2, TC], F32, tag="Pab", name="Pab",
                                 bufs=2)
                for gate in range(3):
                    c0 = gate * HS + ht * 128
                    dst = Pc[:, :tw] if gate == 2 else Pab[:, gate, :tw]
                    nterm = 3 if gate == 2 else 1
                    step = 0
                    for term in range(nterm):
                        for p in range(NPAIR):
                            if term == 1:       # x8 @ Wlo8 (c gate only)
                                stat = wlt[:, p, :, ht * 128:ht * 128 + 128]
                                mov = xall[:, 2 * p:2 * p + 2, csl]
                            elif term == 2:     # xlo8 @ W8 (c gate only)
                                stat = wt[:, p, :, c0:c0 + 128]
                                mov = xall[:, 4 + 2 * p:4 + 2 * p + 2, csl]
                            else:               # x8 @ W8
                                stat = wt[:, p, :, c0:c0 + 128]
                                mov = xall[:, 2 * p:2 * p + 2, csl]
                            nc.tensor.matmul(
                                dst, stat, mov,
                                start=(step == 0),
                                stop=(step == 2 * nterm - 1),
                                perf_mode=PM.DoubleRow)
                            step += 1
                # sigmoid-c first (g on DVE consumes it), then sigmoid-ab
                sg = gpool.tile([128, TC], F16, tag="sg", name="sg")[:, :tw]
                nc.scalar.activation(sg, Pc[:, :tw], AF.Sigmoid)
                sab = gpool.tile([128, 2, TC], F16, tag="sab", name="sab")
                nc.scalar.activation(sab[:, :, :tw], Pab[:, :, :tw],
                                     AF.Sigmoid)
                # g = (pc + 0.5) max sg on DVE; last reader of Pc
                g = gpool.tile([128, TC], F16, tag="g", name="g")[:, :tw]
                nc.vector.scalar_tensor_tensor(g, Pc[:, :tw], 0.5, sg,
                                               OP.add, OP.max)
                # f = sa/(sa+sb) fused on DVE
                f = gpool.tile([128, TC], F16, tag="f", name="f")[:, :tw]
                nc.vector._custom_dve(_GATE_FRAC_OP, out=f,
                                      in0=sab[:, 0, :tw], in1=sab[:, 1, :tw],
                                      s0=0.0, s1=GATE_FRAC_CONSTS["s1"],
                                      imm2=GATE_FRAC_CONSTS["imm2"])
                # emit the PREVIOUS unit's tail (w, v, scan, out-dma) now
                # that this unit's head is queued ahead of it
                flush_pending()
                h = ht_tile[:, ht, :tw]
                carry_ap = ht_tile[:, ht, tw - 1:tw]
                pendq.append((f, g, h, carry_ap, ht, ci, tw, it))
                it += 1
        flush_pending(force=True)
    nc.compile()
    return nc


def _get_nc(mm_mode=MM_MODE):
    if mm_mode not in _nc_cache:
        _nc_cache[mm_mode] = _build_nc(mm_mode)
    return _nc_cache[mm_mode]


def _g_host(x):
    # exp(log_g(x)) of the reference, computed directly in fp32
    return np.where(x >= 0, x + 0.5, 1.0 / (1.0 + np.exp(-np.minimum(x, 0))))


def _prep_x(xb):
    """xb [T, D] f32 -> [NCH*128, 8, TC] e4m3 (x8 + xlo8 k-tiles)."""
    xt = np.ascontiguousarray(xb.reshape(NCH, TC, D).transpose(0, 2, 1))
    x8 = xt.astype(E4)                                   # [NCH, D, TC]
    xlo8 = (xt - x8.astype(np.float32)).astype(E4)
    # [NCH, 8, 128, TC] slot-major -> [NCH, 128, 8, TC]
    both = np.concatenate([x8.reshape(NCH, 4, 128, TC),
                           xlo8.reshape(NCH, 4, 128, TC)], axis=1)
    return np.ascontiguousarray(both.transpose(0, 2, 1, 3)).reshape(
        NCH * 128, 8, TC)


def _prep_w(wcat):
    """[D, C] f32 -> ([128, NPAIR, 2, C] e4m3 base, same-shape lo residual
    of the last HS columns)."""
    w8 = wcat.astype(E4)
    dev = np.ascontiguousarray(
        w8.reshape(NPAIR, 2, 128, wcat.shape[1]).transpose(2, 0, 1, 3))
    return dev


def _run(inputs, mm_mode=MM_MODE, trace=False):
    x = np.asarray(inputs["x"], np.float32)
    h_0 = np.asarray(inputs["h_0"], np.float32)
    W_f = np.asarray(inputs["W_f"], np.float32)
    b_f = np.asarray(inputs["b_f"], np.float32)
    W_i = np.asarray(inputs["W_i"], np.float32)
    b_i = np.asarray(inputs["b_i"], np.float32)
    W_h = np.asarray(inputs["W_h"], np.float32)
    b_h = np.asarray(inputs["b_h"], np.float32)
    assert (b_f == 0).all() and (b_i == 0).all() and (b_h == 0).all(), \
        "device program folds zero biases"

    g0 = _g_host(h_0[:, 0, :])  # [B, H]
    xqs = [_prep_x(x[b]) for b in range(B)]

    in_maps = []
    for c in range(NCORES):
        b, hh = divmod(c, 2)
        hs = slice(hh * HS, (hh + 1) * HS)
        wcat = np.concatenate([W_f[:, hs], W_i[:, hs], W_h[:, hs]], axis=1)
        w8dev = _prep_w(wcat)
        wh = W_h[:, hs]
        whlo = (wh - wh.astype(E4).astype(np.float32))
        wlodev = np.ascontiguousarray(
            whlo.astype(E4).reshape(NPAIR, 2, 128, HS).transpose(2, 0, 1, 3))
        auxa = np.ascontiguousarray(
            g0[b, hs].reshape(NHT, 128).T.astype(np.float32))
        in_maps.append({"xq": xqs[b], "w8": w8dev, "wlo": wlodev,
                        "aux": auxa})

    nc = _get_nc(mm_mode)
    res = run_bass_kernel_spmd(nc, in_maps, core_ids=list(range(NCORES)),
                               trace=trace)

    out = np.empty((B, T + 1, H), np.float32)
    out[:, 0, :] = g0
    for c in range(NCORES):
        b, hh = divmod(c, 2)
        hs = slice(hh * HS, (hh + 1) * HS)
        blk = res.results[c]["out"].astype(np.float32)   # [128, NHT, T]
        out[b, 1:, hs] = blk.transpose(2, 1, 0).reshape(T, HS)
    return out, res


def kernel(**inputs):
    out, _ = _run(inputs)
    return out


# revision 26
# speedup vs baseline: 1.0374x; 1.0072x over previous
"""MinLSTM Trainium2 kernel (fp8 DoubleRow edition).

Full-input contract: kernel(**inputs) takes the complete (unsharded) numpy
inputs of the reference model and returns the full [B, T+1, H] float32 output.

Math (per batch b, channel h — identical to the reference's log-space scan,
computed in linear space; every quantity is positive so the linear recurrence
is numerically stable):
    a = x @ W_f ;  b = x @ W_i ;  c = x @ W_h          (zero biases folded)
    f = sigmoid(a) / (sigmoid(a) + sigmoid(b))          # forget gate
    i = 1 - f                                           # input gate
    g = max(c + 0.5, sigmoid(c))                        # = exp(log_g(c))
    h_t = f_t * h_{t-1} + i_t * g_t,   h_{-1} = g(h_0)
    out[:, 0] = g(h_0); out[:, t+1] = h_t

Sharding: 8 cores, core c -> (sample b = c//2, H-half hh = c%2, 256 channels).
Fully independent cores, no collectives; host assembles the output.

Device pipeline per 512-wide T-chunk, per 128-channel tile:
  - PE: fp8 e4m3 DoubleRow matmuls (0.5 cyc/row, 2 k-tiles per instruction).
    The f/i gates use plain x8@W8; the c gate adds two error-feedback terms
    (x8@(W-W8)8 + (x-x8)8@W8) which cut the end-to-end quantization error
    from ~2.0e-2 to ~0.9e-2 normwise.
  - Act: one Sigmoid instruction over the 3-gate PSUM tile [128, 3, TC].
  - DVE: fused custom op f = sa*~recip(sa+sb) (7 ALU stages, one pass);
    g = (pc + 0.5) max sg (scalar_tensor_tensor); tensor_tensor_scan.
  - Pool: v = (1-f)*g multiply; most of w = 1-f (ones - f subtract).
  - A fraction of w ops go to Act (Copy, scale=-1, bias=1) to balance.
Outputs stored f16 (DMA halved), upcast on host.
"""

from contextlib import ExitStack

import numpy as np
import ml_dtypes

import concourse.bacc as bacc
import concourse.tile as tile
import concourse.mybir as mybir
from concourse.bass_utils import run_bass_kernel_spmd

import concourse.dve_ops as _dve_ops
from concourse.dve_spec import (Spec as _Spec, Src0 as _S0, Src1 as _S1,
                                C1 as _C1, C2 as _C2,
                                AluOp as _AluOp, Bin as _Bin, lower as _lower)
from concourse.dve_uop import DveOpSpec as _DveOpSpec
from concourse.dve_table_gen import dve_ver_for as _dve_ver_for

# ---- fused custom DVE op: f = in0 * ~1/(in0+in1) ---------------------------
# x = in0+in1; nx = bitcast(~x) (exponent-flip reciprocal seed); u = x*nx
# lands in [-4.5,-4]; f = in0 * nx * (c1*u + c2), deg-1 minimax of 1/u on
# that interval (max rel err ~2.2e-3). 7 ALU stages -> single DVE pass.
GATE_FRAC_CONSTS = {"s1": -0.0546648, "imm2": -0.4681172}


def _register_gate_frac():
    name = "GATE_FRAC_ANT"
    if name in _dve_ops._SUB_OPCODE_FOR_NAME:
        return next(o for o in _dve_ops.OPS if o.name == name)
    _x = _S0 + _S1
    _nx = _Bin(_AluOp.BITWISE_NOT, _x, _x)
    _u = _x * _nx

    def _ref(in0, in1, c0, c1, c2):
        in0 = np.asarray(in0, np.float32)
        in1 = np.asarray(in1, np.float32)
        x = (in0 + in1).astype(np.float32)
        nx = (~x.view(np.int32)).view(np.float32)
        u = x * nx
        return (nx * (c1 * u + c2)) * in0

    spec = _Spec(body=(_Bin(_AluOp.MULTIPLY, _nx, (_C1 * _u + _C2))) * _S0,
                 reference=_ref)
    row = _dve_ops._CUSTOM_DVE_ROW_BASE + len(_dve_ops.OPS)
    assert row < 0x20
    ver = _dve_ver_for("TRN2")
    sha = _DveOpSpec(name=name, opcode=row, uops=_lower(spec, ver=ver),
                     rd1_en=True).sha(ver)
    op = _dve_ops.DveOp(name, spec, subdim=False, uops_sha={ver: sha})
    _dve_ops.OPS.append(op)
    _dve_ops.CUSTOM_DVE_SPECS[name] = spec
    _dve_ops._SUB_OPCODE_FOR_NAME[name] = row
    return op


_GATE_FRAC_OP = _register_gate_frac()

F32 = mybir.dt.float32
F16 = mybir.dt.float16
F8 = mybir.dt.float8e4
AF = mybir.ActivationFunctionType
OP = mybir.AluOpType
PM = mybir.MatmulPerfMode
E4 = ml_dtypes.float8_e4m3fn

B, T, D, H = 4, 8192, 512, 512
NCORES = 8
HS = H // 2          # 256 channels per core
TC = 512             # T chunk width
NCH = T // TC        # 16 chunks
NHT = HS // 128      # 2 h-tiles per core
NPAIR = 2            # DoubleRow k-tile pairs covering D=512

MM_MODE = "fp8dr"    # kept for test.py compatibility

# every W_ACT_EVERY-th (chunk,ht) unit computes w = 1-f on Act instead of
# Pool, balancing Act ~53us / Pool ~60us under the DVE ~63us bottleneck
W_ACT_EVERY = 1   # unused, kept for reference
# w-engine pattern: p=Pool, d=DVE, a=Act (balances the three engines)
W_PATTERN = "ppa"

_nc_cache = {}


def _build_nc(mm_mode=MM_MODE):
    nc = bacc.Bacc("TRN2", target_bir_lowering=False, debug=False,
                   num_devices=NCORES)
    # x8 (slots 0-3) and xlo8 (slots 4-7) k-tiles, chunk-major
    xq = nc.dram_tensor("xq", [NCH * 128, 8, TC], F8, kind="ExternalInput")
    w8 = nc.dram_tensor("w8", [128, NPAIR, 2, 3 * HS], F8,
                        kind="ExternalInput")
    wlo = nc.dram_tensor("wlo", [128, NPAIR, 2, HS], F8,
                         kind="ExternalInput")
    aux = nc.dram_tensor("aux", [128, NHT], F32, kind="ExternalInput")
    out = nc.dram_tensor("out", [128, NHT, T], F16, kind="ExternalOutput")

    with tile.TileContext(nc) as tc, ExitStack() as ctx:
        wpool = ctx.enter_context(tc.tile_pool(name="w", bufs=1))
        xpool = ctx.enter_context(tc.tile_pool(name="x", bufs=4))
        gpool = ctx.enter_context(tc.tile_pool(name="g", bufs=6))
        hpool = ctx.enter_context(tc.tile_pool(name="h", bufs=5))
        ppool = ctx.enter_context(tc.tile_pool(name="p", bufs=2, space="PSUM"))

        # weight/aux loads on the ACT HWDGE queue so the first x-chunk loads
        # (SP queue) run in parallel with them
        wt = wpool.tile([128, NPAIR, 2, 3 * HS], F8, tag="w8", name="wt")
        nc.scalar.dma_start(wt[:], w8[:])
        wlt = wpool.tile([128, NPAIR, 2, HS], F8, tag="wlo", name="wlt")
        nc.scalar.dma_start(wlt[:], wlo[:])
        auxt = wpool.tile([128, NHT], F32, tag="aux", name="auxt")
        nc.scalar.dma_start(auxt[:], aux[:])
        ones = wpool.tile([128, TC], F16, tag="ones", name="ones")
        nc.vector.memset(ones[:], 1.0)

        # PE p-state warmup: the tensor engine needs ~3us of continuous work
        # to reach full clock. The first ~5us of the kernel are DMA-only, so
        # burn them on scratch matmuls (no data dependencies) and the first
        # real matmuls run at full speed.
        wu_s = wpool.tile([128, 2, 128], F8, tag="wu_s", name="wu_s")
        nc.vector.memset(wu_s[:], 0.0)
        wu_m = wpool.tile([128, 2, TC], F8, tag="wu_m", name="wu_m")
        nc.vector.memset(wu_m[:], 0.0)
        wu_p = ppool.tile([128, TC], F32, tag="Pc", name="wu_p", bufs=4)
        for _ in range(12):
            nc.tensor.matmul(wu_p[:], wu_s[:], wu_m[:], start=True,
                             stop=True, perf_mode=PM.DoubleRow)

        # chunk 0 split in half so the pipeline primes on half-size units
        chunks = [(0, TC // 2), (TC // 2, TC // 2)]
        chunks += [(k * TC, TC) for k in range(1, NCH)]

        NUNITS = len(chunks) * NHT
        carry = [None] * NHT
        it = 0
        # two-unit software pipeline: unit k's head [matmuls, sigmoid, g, f]
        # is emitted before unit k-2's tail [w, v, scan, out-dma]. The scan
        # of unit k-2 then never stalls the in-order DVE: its v input has
        # been computable since unit k-1's head ran.
        TAIL_DELAY = 2
        from collections import deque
        pendq = deque()  # (f, g, h_slice, ini, carry_ap, ht, ci, tw, unit#)
        out_dmas = {}    # ci -> (ht_tile, tsl, tw) emitted when ht=1 scanned

        def flush_pending(force=False):
            while pendq and (force or len(pendq) > TAIL_DELAY):
                f_, g_, h_, carry_ap, ht_, ci_, tw_, k_ = pendq.popleft()
                # resolve the scan initial lazily: the same-ht predecessor's
                # scan is emitted exactly TAIL_DELAY units before this flush
                ini_ = auxt[:, ht_:ht_ + 1] if ci_ == 0 else carry[ht_]
                # w = 1-f: Pool / DVE / Act per balance pattern; in the
                # drain (Act idle, Pool the laggard) w goes to Act and v to
                # DVE so the last scans issue as early as possible
                drain = k_ >= NUNITS - 2
                w_ = gpool.tile([128, TC], F16, tag="w", name="w_")[:, :tw_]
                wsel = "a" if drain else W_PATTERN[k_ % len(W_PATTERN)]
                if wsel == "a":
                    nc.scalar.activation(w_, f_, AF.Copy, bias=1.0,
                                         scale=-1.0)
                elif wsel == "d":
                    nc.vector.tensor_scalar(w_, f_, -1.0, 1.0,
                                            OP.mult, OP.add)
                else:
                    nc.gpsimd.tensor_tensor(w_, ones[:, :tw_], f_,
                                            op=OP.subtract)
                v_ = gpool.tile([128, TC], F16, tag="v", name="v_")[:, :tw_]
                if drain:
                    nc.vector.tensor_tensor(v_, w_, g_, op=OP.mult)
                else:
                    nc.gpsimd.tensor_tensor(v_, w_, g_, op=OP.mult)
                nc.vector.tensor_tensor_scan(h_, f_, v_, ini_, OP.mult,
                                             OP.add)
                carry[ht_] = carry_ap
                if ci_ == len(chunks) - 1:
                    htile_, tsl_, twc_ = out_dmas[ci_]
                    nc.sync.dma_start(out[:, ht_:ht_ + 1, tsl_],
                                      htile_[:, ht_:ht_ + 1, :twc_])
                elif ht_ == NHT - 1:
                    htile_, tsl_, twc_ = out_dmas.pop(ci_)
                    nc.sync.dma_start(out[:, :, tsl_], htile_[:, :, :twc_])

        for ci, (t0, tw) in enumerate(chunks):
            tsl = slice(t0, t0 + tw)
            xall = xpool.tile([128, 8, TC], F8, tag="xall", name="xall")
            crow = (t0 // TC) * 128
            csl = slice(t0 % TC, t0 % TC + tw)
            if ci == 0:
                # first chunk: land the x8 k-tiles (slots 0-3) before the
                # xlo slots so the f/i matmuls can start sooner
                nc.sync.dma_start(xall[:, :4, csl.start:csl.stop],
                                  xq[crow:crow + 128, :4, csl])
                nc.sync.dma_start(xall[:, 4:, csl.start:csl.stop],
                                  xq[crow:crow + 128, 4:, csl])
            else:
                nc.sync.dma_start(xall[:, :, csl.start:csl.stop] if tw != TC
                                  else xall[:],
                                  xq[crow:crow + 128, :, csl])
            ht_tile = hpool.tile([128, NHT, TC], F16, tag="h", name="ht_tile")
            out_dmas[ci] = (ht_tile, tsl, tw)
            for ht in range(NHT):
                # split PSUM: Pc (1 bank, 4-deep rotation — its late reader
                # g never gates the PE) and Pab (2 banks, 2-deep — freed by
                # sigmoid-ab alone)
                Pc = ppool.tile([128, TC], F32, tag="Pc", name="Pc", bufs=4)
                Pab = ppool.tile([128, 2, TC], F32, tag="Pab", name="Pab",
                                 bufs=2)
                for gate in range(3):
                    c0 = gate * HS + ht * 128
                    dst = Pc[:, :tw] if gate == 2 else Pab[:, gate, :tw]
                    nterm = 3 if gate == 2 else 1
                    step = 0
                    for term in range(nterm):
                        for p in range(NPAIR):
                            if term == 1:       # x8 @ Wlo8 (c gate only)
                                stat = wlt[:, p, :, ht * 128:ht * 128 + 128]
                                mov = xall[:, 2 * p:2 * p + 2, csl]
                            elif term == 2:     # xlo8 @ W8 (c gate only)
                                stat = wt[:, p, :, c0:c0 + 128]
                                mov = xall[:, 4 + 2 * p:4 + 2 * p + 2, csl]
                            else:               # x8 @ W8
                                stat = wt[:, p, :, c0:c0 + 128]
                                mov = xall[:, 2 * p:2 * p + 2, csl]
                            nc.tensor.matmul(
                                dst, stat, mov,
                                start=(step == 0),
                                stop=(step == 2 * nterm - 1),
                                perf_mode=PM.DoubleRow)
                            step += 1
                # sigmoid-c first (g on DVE consumes it), then sigmoid-ab
                sg = gpool.tile([128, TC], F16, tag="sg", name="sg")[:, :tw]
                nc.scalar.activation(sg, Pc[:, :tw], AF.Sigmoid)
                sab = gpool.tile([128, 2, TC], F16, tag="sab", name="sab")
                nc.scalar.activation(sab[:, :, :tw], Pab[:, :, :tw],
                                     AF.Sigmoid)
                # g = (pc + 0.5) max sg on DVE; last reader of Pc
                g = gpool.tile([128, TC], F16, tag="g", name="g")[:, :tw]
                nc.vector.scalar_tensor_tensor(g, Pc[:, :tw], 0.5, sg,
                                               OP.add, OP.max)
                # f = sa/(sa+sb) fused on DVE
                f = gpool.tile([128, TC], F16, tag="f", name="f")[:, :tw]
                nc.vector._custom_dve(_GATE_FRAC_OP, out=f,
                                      in0=sab[:, 0, :tw], in1=sab[:, 1, :tw],
                                      s0=0.0, s1=GATE_FRAC_CONSTS["s1"],
                                      imm2=GATE_FRAC_CONSTS["imm2"])
                # emit the PREVIOUS unit's tail (w, v, scan, out-dma) now
                # that this unit's head is queued ahead of it
                flush_pending()
                h = ht_tile[:, ht, :tw]
                carry_ap = ht_tile[:, ht, tw - 1:tw]
                pendq.append((f, g, h, carry_ap, ht, ci, tw, it))
                it += 1
        flush_pending(force=True)
    nc.compile()
    return nc


def _get_nc(mm_mode=MM_MODE):
    if mm_mode not in _nc_cache:
        _nc_cache[mm_mode] = _build_nc(mm_mode)
    return _nc_cache[mm_mode]


def _g_host(x):
    # exp(log_g(x)) of the reference, computed directly in fp32
    return np.where(x >= 0, x + 0.5, 1.0 / (1.0 + np.exp(-np.minimum(x, 0))))


def _prep_x(xb):
    """xb [T, D] f32 -> [NCH*128, 8, TC] e4m3 (x8 + xlo8 k-tiles)."""
    xt = np.ascontiguousarray(xb.reshape(NCH, TC, D).transpose(0, 2, 1))
    x8 = xt.astype(E4)                                   # [NCH, D, TC]
    xlo8 = (xt - x8.astype(np.float32)).astype(E4)
    # [NCH, 8, 128, TC] slot-major -> [NCH, 128, 8, TC]
    both = np.concatenate([x8.reshape(NCH, 4, 128, TC),
                           xlo8.reshape(NCH, 4, 128, TC)], axis=1)
    return np.ascontiguousarray(both.transpose(0, 2, 1, 3)).reshape(
        NCH * 128, 8, TC)


def _prep_w(wcat):
    """[D, C] f32 -> ([128, NPAIR, 2, C] e4m3 base, same-shape lo residual
    of the last HS columns)."""
    w8 = wcat.astype(E4)
    dev = np.ascontiguousarray(
        w8.reshape(NPAIR, 2, 128, wcat.shape[1]).transpose(2, 0, 1, 3))
    return dev


def _run(inputs, mm_mode=MM_MODE, trace=False):
    x = np.asarray(inputs["x"], np.float32)
    h_0 = np.asarray(inputs["h_0"], np.float32)
    W_f = np.asarray(inputs["W_f"], np.float32)
    b_f = np.asarray(inputs["b_f"], np.float32)
    W_i = np.asarray(inputs["W_i"], np.float32)
    b_i = np.asarray(inputs["b_i"], np.float32)
    W_h = np.asarray(inputs["W_h"], np.float32)
    b_h = np.asarray(inputs["b_h"], np.float32)
    assert (b_f == 0).all() and (b_i == 0).all() and (b_h == 0).all(), \
        "device program folds zero biases"

    g0 = _g_host(h_0[:, 0, :])  # [B, H]
    xqs = [_prep_x(x[b]) for b in range(B)]

    in_maps = []
    for c in range(NCORES):
        b, hh = divmod(c, 2)
        hs = slice(hh * HS, (hh + 1) * HS)
        wcat = np.concatenate([W_f[:, hs], W_i[:, hs], W_h[:, hs]], axis=1)
        w8dev = _prep_w(wcat)
        wh = W_h[:, hs]
        whlo = (wh - wh.astype(E4).astype(np.float32))
        wlodev = np.ascontiguousarray(
            whlo.astype(E4).reshape(NPAIR, 2, 128, HS).transpose(2, 0, 1, 3))
        auxa = np.ascontiguousarray(
            g0[b, hs].reshape(NHT, 128).T.astype(np.float32))
        in_maps.append({"xq": xqs[b], "w8": w8dev, "wlo": wlodev,
                        "aux": auxa})

    nc = _get_nc(mm_mode)
    res = run_bass_kernel_spmd(nc, in_maps, core_ids=list(range(NCORES)),
                               trace=trace)

    out = np.empty((B, T + 1, H), np.float32)
    out[:, 0, :] = g0
    for c in range(NCORES):
        b, hh = divmod(c, 2)
        hs = slice(hh * HS, (hh + 1) * HS)
        blk = res.results[c]["out"].astype(np.float32)   # [128, NHT, T]
        out[b, 1:, hs] = blk.transpose(2, 1, 0).reshape(T, HS)
    return out, res


def kernel(**inputs):
    out, _ = _run(inputs)
    return out


# revision 27
# speedup vs baseline: 1.0391x; 1.0016x over previous
"""MinLSTM Trainium2 kernel (fp8 DoubleRow edition).

Full-input contract: kernel(**inputs) takes the complete (unsharded) numpy
inputs of the reference model and returns the full [B, T+1, H] float32 output.

Math (per batch b, channel h — identical to the reference's log-space scan,
computed in linear space; every quantity is positive so the linear recurrence
is numerically stable):
    a = x @ W_f ;  b = x @ W_i ;  c = x @ W_h          (zero biases folded)
    f = sigmoid(a) / (sigmoid(a) + sigmoid(b))          # forget gate
    i = 1 - f                                           # input gate
    g = max(c + 0.5, sigmoid(c))                        # = exp(log_g(c))
    h_t = f_t * h_{t-1} + i_t * g_t,   h_{-1} = g(h_0)
    out[:, 0] = g(h_0); out[:, t+1] = h_t

Sharding: 8 cores, core c -> (sample b = c//2, H-half hh = c%2, 256 channels).
Fully independent cores, no collectives; host assembles the output.

Device pipeline per 512-wide T-chunk, per 128-channel tile:
  - PE: fp8 e4m3 DoubleRow matmuls (0.5 cyc/row, 2 k-tiles per instruction).
    The f/i gates use plain x8@W8; the c gate adds two error-feedback terms
    (x8@(W-W8)8 + (x-x8)8@W8) which cut the end-to-end quantization error
    from ~2.0e-2 to ~0.9e-2 normwise.
  - Act: one Sigmoid instruction over the 3-gate PSUM tile [128, 3, TC].
  - DVE: fused custom op f = sa*~recip(sa+sb) (7 ALU stages, one pass);
    g = (pc + 0.5) max sg (scalar_tensor_tensor); tensor_tensor_scan.
  - Pool: v = (1-f)*g multiply; most of w = 1-f (ones - f subtract).
  - A fraction of w ops go to Act (Copy, scale=-1, bias=1) to balance.
Outputs stored f16 (DMA halved), upcast on host.
"""

from contextlib import ExitStack

import numpy as np
import ml_dtypes

import concourse.bacc as bacc
import concourse.tile as tile
import concourse.mybir as mybir
from concourse.bass_utils import run_bass_kernel_spmd

import concourse.dve_ops as _dve_ops
from concourse.dve_spec import (Spec as _Spec, Src0 as _S0, Src1 as _S1,
                                C1 as _C1, C2 as _C2,
                                AluOp as _AluOp, Bin as _Bin, lower as _lower)
from concourse.dve_uop import DveOpSpec as _DveOpSpec
from concourse.dve_table_gen import dve_ver_for as _dve_ver_for

# ---- fused custom DVE op: f = in0 * ~1/(in0+in1) ---------------------------
# x = in0+in1; nx = bitcast(~x) (exponent-flip reciprocal seed); u = x*nx
# lands in [-4.5,-4]; f = in0 * nx * (c1*u + c2), deg-1 minimax of 1/u on
# that interval (max rel err ~2.2e-3). 7 ALU stages -> single DVE pass.
GATE_FRAC_CONSTS = {"s1": -0.0546648, "imm2": -0.4681172}


def _register_gate_frac():
    name = "GATE_FRAC_ANT"
    if name in _dve_ops._SUB_OPCODE_FOR_NAME:
        return next(o for o in _dve_ops.OPS if o.name == name)
    _x = _S0 + _S1
    _nx = _Bin(_AluOp.BITWISE_NOT, _x, _x)
    _u = _x * _nx

    def _ref(in0, in1, c0, c1, c2):
        in0 = np.asarray(in0, np.float32)
        in1 = np.asarray(in1, np.float32)
        x = (in0 + in1).astype(np.float32)
        nx = (~x.view(np.int32)).view(np.float32)
        u = x * nx
        return (nx * (c1 * u + c2)) * in0

    spec = _Spec(body=(_Bin(_AluOp.MULTIPLY, _nx, (_C1 * _u + _C2))) * _S0,
                 reference=_ref)
    row = _dve_ops._CUSTOM_DVE_ROW_BASE + len(_dve_ops.OPS)
    assert row < 0x20
    ver = _dve_ver_for("TRN2")
    sha = _DveOpSpec(name=name, opcode=row, uops=_lower(spec, ver=ver),
                     rd1_en=True).sha(ver)
    op = _dve_ops.DveOp(name, spec, subdim=False, uops_sha={ver: sha})
    _dve_ops.OPS.append(op)
    _dve_ops.CUSTOM_DVE_SPECS[name] = spec
    _dve_ops._SUB_OPCODE_FOR_NAME[name] = row
    return op


_GATE_FRAC_OP = _register_gate_frac()

F32 = mybir.dt.float32
F16 = mybir.dt.float16
F8 = mybir.dt.float8e4
AF = mybir.ActivationFunctionType
OP = mybir.AluOpType
PM = mybir.MatmulPerfMode
E4 = ml_dtypes.float8_e4m3fn

B, T, D, H = 4, 8192, 512, 512
NCORES = 8
HS = H // 2          # 256 channels per core
TC = 512             # T chunk width
NCH = T // TC        # 16 chunks
NHT = HS // 128      # 2 h-tiles per core
NPAIR = 2            # DoubleRow k-tile pairs covering D=512

MM_MODE = "fp8dr"    # kept for test.py compatibility

# every W_ACT_EVERY-th (chunk,ht) unit computes w = 1-f on Act instead of
# Pool, balancing Act ~53us / Pool ~60us under the DVE ~63us bottleneck
W_ACT_EVERY = 1   # unused, kept for reference
# w-engine pattern: p=Pool, d=DVE, a=Act (balances the three engines)
W_PATTERN = "ppa"

_nc_cache = {}


def _build_nc(mm_mode=MM_MODE):
    nc = bacc.Bacc("TRN2", target_bir_lowering=False, debug=False,
                   num_devices=NCORES)
    # x8 (slots 0-3) and xlo8 (slots 4-7) k-tiles, chunk-major
    xq = nc.dram_tensor("xq", [NCH * 128, 8, TC], F8, kind="ExternalInput")
    w8 = nc.dram_tensor("w8", [128, NPAIR, 2, 3 * HS], F8,
                        kind="ExternalInput")
    wlo = nc.dram_tensor("wlo", [128, NPAIR, 2, HS], F8,
                         kind="ExternalInput")
    aux = nc.dram_tensor("aux", [128, NHT], F32, kind="ExternalInput")
    out = nc.dram_tensor("out", [128, NHT, T], F16, kind="ExternalOutput")

    with tile.TileContext(nc) as tc, ExitStack() as ctx:
        wpool = ctx.enter_context(tc.tile_pool(name="w", bufs=1))
        xpool = ctx.enter_context(tc.tile_pool(name="x", bufs=4))
        gpool = ctx.enter_context(tc.tile_pool(name="g", bufs=6))
        hpool = ctx.enter_context(tc.tile_pool(name="h", bufs=5))
        ppool = ctx.enter_context(tc.tile_pool(name="p", bufs=2, space="PSUM"))

        # weight/aux loads on the ACT HWDGE queue so the first x-chunk loads
        # (SP queue) run in parallel with them
        wt = wpool.tile([128, NPAIR, 2, 3 * HS], F8, tag="w8", name="wt")
        nc.scalar.dma_start(wt[:], w8[:])
        wlt = wpool.tile([128, NPAIR, 2, HS], F8, tag="wlo", name="wlt")
        nc.scalar.dma_start(wlt[:], wlo[:])
        auxt = wpool.tile([128, NHT], F32, tag="aux", name="auxt")
        nc.scalar.dma_start(auxt[:], aux[:])
        ones = wpool.tile([128, TC], F16, tag="ones", name="ones")
        nc.vector.memset(ones[:], 1.0)

        # trigger the Act function-table loads immediately (they otherwise
        # land right before the first sigmoid and delay it)
        dummy = wpool.tile([128, 1], F16, tag="dummy", name="dummy")
        nc.scalar.activation(dummy[:], ones[:, 0:1], AF.Sigmoid)
        nc.scalar.activation(dummy[:], ones[:, 0:1], AF.Copy, bias=1.0,
                             scale=-1.0)

        # PE p-state warmup: the tensor engine needs ~3us of continuous work
        # to reach full clock. The first ~5us of the kernel are DMA-only, so
        # burn them on scratch matmuls (no data dependencies) and the first
        # real matmuls run at full speed.
        wu_s = wpool.tile([128, 2, 128], F8, tag="wu_s", name="wu_s")
        nc.vector.memset(wu_s[:], 0.0)
        wu_m = wpool.tile([128, 2, TC], F8, tag="wu_m", name="wu_m")
        nc.vector.memset(wu_m[:], 0.0)
        wu_p = ppool.tile([128, TC], F32, tag="Pc", name="wu_p", bufs=4)
        for _ in range(12):
            nc.tensor.matmul(wu_p[:], wu_s[:], wu_m[:], start=True,
                             stop=True, perf_mode=PM.DoubleRow)

        # chunk 0 split in half so the pipeline primes on half-size units
        chunks = [(0, TC // 2), (TC // 2, TC // 2)]
        chunks += [(k * TC, TC) for k in range(1, NCH)]

        NUNITS = len(chunks) * NHT
        carry = [None] * NHT
        it = 0
        # two-unit software pipeline: unit k's head [matmuls, sigmoid, g, f]
        # is emitted before unit k-2's tail [w, v, scan, out-dma]. The scan
        # of unit k-2 then never stalls the in-order DVE: its v input has
        # been computable since unit k-1's head ran.
        TAIL_DELAY = 2
        from collections import deque
        pendq = deque()  # (f, g, h_slice, ini, carry_ap, ht, ci, tw, unit#)
        out_dmas = {}    # ci -> (ht_tile, tsl, tw) emitted when ht=1 scanned

        def flush_pending(force=False):
            while pendq and (force or len(pendq) > TAIL_DELAY):
                f_, g_, h_, carry_ap, ht_, ci_, tw_, k_ = pendq.popleft()
                # resolve the scan initial lazily: the same-ht predecessor's
                # scan is emitted exactly TAIL_DELAY units before this flush
                ini_ = auxt[:, ht_:ht_ + 1] if ci_ == 0 else carry[ht_]
                # w = 1-f: Pool / DVE / Act per balance pattern; in the
                # drain (Act idle, Pool the laggard) w goes to Act and v to
                # DVE so the last scans issue as early as possible
                drain = k_ >= NUNITS - 2
                w_ = gpool.tile([128, TC], F16, tag="w", name="w_")[:, :tw_]
                wsel = "a" if drain else W_PATTERN[k_ % len(W_PATTERN)]
                if wsel == "a":
                    nc.scalar.activation(w_, f_, AF.Copy, bias=1.0,
                                         scale=-1.0)
                elif wsel == "d":
                    nc.vector.tensor_scalar(w_, f_, -1.0, 1.0,
                                            OP.mult, OP.add)
                else:
                    nc.gpsimd.tensor_tensor(w_, ones[:, :tw_], f_,
                                            op=OP.subtract)
                v_ = gpool.tile([128, TC], F16, tag="v", name="v_")[:, :tw_]
                if drain:
                    nc.vector.tensor_tensor(v_, w_, g_, op=OP.mult)
                else:
                    nc.gpsimd.tensor_tensor(v_, w_, g_, op=OP.mult)
                nc.vector.tensor_tensor_scan(h_, f_, v_, ini_, OP.mult,
                                             OP.add)
                carry[ht_] = carry_ap
                if ci_ == len(chunks) - 1:
                    htile_, tsl_, twc_ = out_dmas[ci_]
                    nc.sync.dma_start(out[:, ht_:ht_ + 1, tsl_],
                                      htile_[:, ht_:ht_ + 1, :twc_])
                elif ht_ == NHT - 1:
                    htile_, tsl_, twc_ = out_dmas.pop(ci_)
                    nc.sync.dma_start(out[:, :, tsl_], htile_[:, :, :twc_])

        for ci, (t0, tw) in enumerate(chunks):
            tsl = slice(t0, t0 + tw)
            xall = xpool.tile([128, 8, TC], F8, tag="xall", name="xall")
            crow = (t0 // TC) * 128
            csl = slice(t0 % TC, t0 % TC + tw)
            if ci == 0:
                # first chunk: land the x8 k-tiles (slots 0-3) before the
                # xlo slots so the f/i matmuls can start sooner
                nc.sync.dma_start(xall[:, :4, csl.start:csl.stop],
                                  xq[crow:crow + 128, :4, csl])
                nc.sync.dma_start(xall[:, 4:, csl.start:csl.stop],
                                  xq[crow:crow + 128, 4:, csl])
            else:
                nc.sync.dma_start(xall[:, :, csl.start:csl.stop] if tw != TC
                                  else xall[:],
                                  xq[crow:crow + 128, :, csl])
            ht_tile = hpool.tile([128, NHT, TC], F16, tag="h", name="ht_tile")
            out_dmas[ci] = (ht_tile, tsl, tw)
            for ht in range(NHT):
                # split PSUM: Pc (1 bank, 4-deep rotation — its late reader
                # g never gates the PE) and Pab (2 banks, 2-deep — freed by
                # sigmoid-ab alone)
                Pc = ppool.tile([128, TC], F32, tag="Pc", name="Pc", bufs=4)
                Pab = ppool.tile([128, 2, TC], F32, tag="Pab", name="Pab",
                                 bufs=2)
                for gate in range(3):
                    c0 = gate * HS + ht * 128
                    dst = Pc[:, :tw] if gate == 2 else Pab[:, gate, :tw]
                    nterm = 3 if gate == 2 else 1
                    step = 0
                    for term in range(nterm):
                        for p in range(NPAIR):
                            if term == 1:       # x8 @ Wlo8 (c gate only)
                                stat = wlt[:, p, :, ht * 128:ht * 128 + 128]
                                mov = xall[:, 2 * p:2 * p + 2, csl]
                            elif term == 2:     # xlo8 @ W8 (c gate only)
                                stat = wt[:, p, :, c0:c0 + 128]
                                mov = xall[:, 4 + 2 * p:4 + 2 * p + 2, csl]
                            else:               # x8 @ W8
                                stat = wt[:, p, :, c0:c0 + 128]
                                mov = xall[:, 2 * p:2 * p + 2, csl]
                            nc.tensor.matmul(
                                dst, stat, mov,
                                start=(step == 0),
                                stop=(step == 2 * nterm - 1),
                                perf_mode=PM.DoubleRow)
                            step += 1
                # sigmoid-c first (g on DVE consumes it), then sigmoid-ab
                sg = gpool.tile([128, TC], F16, tag="sg", name="sg")[:, :tw]
                nc.scalar.activation(sg, Pc[:, :tw], AF.Sigmoid)
                sab = gpool.tile([128, 2, TC], F16, tag="sab", name="sab")
                nc.scalar.activation(sab[:, :, :tw], Pab[:, :, :tw],
                                     AF.Sigmoid)
                # g = (pc + 0.5) max sg on DVE; last reader of Pc
                g = gpool.tile([128, TC], F16, tag="g", name="g")[:, :tw]
                nc.vector.scalar_tensor_tensor(g, Pc[:, :tw], 0.5, sg,
                                               OP.add, OP.max)
                # f = sa/(sa+sb) fused on DVE
                f = gpool.tile([128, TC], F16, tag="f", name="f")[:, :tw]
                nc.vector._custom_dve(_GATE_FRAC_OP, out=f,
                                      in0=sab[:, 0, :tw], in1=sab[:, 1, :tw],
                                      s0=0.0, s1=GATE_FRAC_CONSTS["s1"],
                                      imm2=GATE_FRAC_CONSTS["imm2"])
                # emit the PREVIOUS unit's tail (w, v, scan, out-dma) now
                # that this unit's head is queued ahead of it
                flush_pending()
                h = ht_tile[:, ht, :tw]
                carry_ap = ht_tile[:, ht, tw - 1:tw]
                pendq.append((f, g, h, carry_ap, ht, ci, tw, it))
                it += 1
        flush_pending(force=True)
    nc.compile()
    return nc


def _get_nc(mm_mode=MM_MODE):
    if mm_mode not in _nc_cache:
        _nc_cache[mm_mode] = _build_nc(mm_mode)
    return _nc_cache[mm_mode]


def _g_host(x):
    # exp(log_g(x)) of the reference, computed directly in fp32
    return np.where(x >= 0, x + 0.5, 1.0 / (1.0 + np.exp(-np.minimum(x, 0))))


def _prep_x(xb):
    """xb [T, D] f32 -> [NCH*128, 8, TC] e4m3 (x8 + xlo8 k-tiles)."""
    xt = np.ascontiguousarray(xb.reshape(NCH, TC, D).transpose(0, 2, 1))
    x8 = xt.astype(E4)                                   # [NCH, D, TC]
    xlo8 = (xt - x8.astype(np.float32)).astype(E4)
    # [NCH, 8, 128, TC] slot-major -> [NCH, 128, 8, TC]
    both = np.concatenate([x8.reshape(NCH, 4, 128, TC),
                           xlo8.reshape(NCH, 4, 128, TC)], axis=1)
    return np.ascontiguousarray(both.transpose(0, 2, 1, 3)).reshape(
        NCH * 128, 8, TC)


def _prep_w(wcat):
    """[D, C] f32 -> ([128, NPAIR, 2, C] e4m3 base, same-shape lo residual
    of the last HS columns)."""
    w8 = wcat.astype(E4)
    dev = np.ascontiguousarray(
        w8.reshape(NPAIR, 2, 128, wcat.shape[1]).transpose(2, 0, 1, 3))
    return dev


def _run(inputs, mm_mode=MM_MODE, trace=False):
    x = np.asarray(inputs["x"], np.float32)
    h_0 = np.asarray(inputs["h_0"], np.float32)
    W_f = np.asarray(inputs["W_f"], np.float32)
    b_f = np.asarray(inputs["b_f"], np.float32)
    W_i = np.asarray(inputs["W_i"], np.float32)
    b_i = np.asarray(inputs["b_i"], np.float32)
    W_h = np.asarray(inputs["W_h"], np.float32)
    b_h = np.asarray(inputs["b_h"], np.float32)
    assert (b_f == 0).all() and (b_i == 0).all() and (b_h == 0).all(), \
        "device program folds zero biases"

    g0 = _g_host(h_0[:, 0, :])  # [B, H]
    xqs = [_prep_x(x[b]) for b in range(B)]

    in_maps = []
    for c in range(NCORES):
        b, hh = divmod(c, 2)
        hs = slice(hh * HS, (hh + 1) * HS)
        wcat = np.concatenate([W_f[:, hs], W_i[:, hs], W_h[:, hs]], axis=1)
        w8dev = _prep_w(wcat)
        wh = W_h[:, hs]
        whlo = (wh - wh.astype(E4).astype(np.float32))
        wlodev = np.ascontiguousarray(
            whlo.astype(E4).reshape(NPAIR, 2, 128, HS).transpose(2, 0, 1, 3))
        auxa = np.ascontiguousarray(
            g0[b, hs].reshape(NHT, 128).T.astype(np.float32))
        in_maps.append({"xq": xqs[b], "w8": w8dev, "wlo": wlodev,
                        "aux": auxa})

    nc = _get_nc(mm_mode)
    res = run_bass_kernel_spmd(nc, in_maps, core_ids=list(range(NCORES)),
                               trace=trace)

    out = np.empty((B, T + 1, H), np.float32)
    out[:, 0, :] = g0
    for c in range(NCORES):
        b, hh = divmod(c, 2)
        hs = slice(hh * HS, (hh + 1) * HS)
        blk = res.results[c]["out"].astype(np.float32)   # [128, NHT, T]
        out[b, 1:, hs] = blk.transpose(2, 1, 0).reshape(T, HS)
    return out, res


def kernel(**inputs):
    out, _ = _run(inputs)
    return out


# revision 28
# speedup vs baseline: 1.0414x; 1.0022x over previous
"""MinLSTM Trainium2 kernel (fp8 DoubleRow edition).

Full-input contract: kernel(**inputs) takes the complete (unsharded) numpy
inputs of the reference model and returns the full [B, T+1, H] float32 output.

Math (per batch b, channel h — identical to the reference's log-space scan,
computed in linear space; every quantity is positive so the linear recurrence
is numerically stable):
    a = x @ W_f ;  b = x @ W_i ;  c = x @ W_h          (zero biases folded)
    f = sigmoid(a) / (sigmoid(a) + sigmoid(b))          # forget gate
    i = 1 - f                                           # input gate
    g = max(c + 0.5, sigmoid(c))                        # = exp(log_g(c))
    h_t = f_t * h_{t-1} + i_t * g_t,   h_{-1} = g(h_0)
    out[:, 0] = g(h_0); out[:, t+1] = h_t

Sharding: 8 cores, core c -> (sample b = c//2, H-half hh = c%2, 256 channels).
Fully independent cores, no collectives; host assembles the output.

Device pipeline per 512-wide T-chunk, per 128-channel tile:
  - PE: fp8 e4m3 DoubleRow matmuls (0.5 cyc/row, 2 k-tiles per instruction).
    The f/i gates use plain x8@W8; the c gate adds two error-feedback terms
    (x8@(W-W8)8 + (x-x8)8@W8) which cut the end-to-end quantization error
    from ~2.0e-2 to ~0.9e-2 normwise.
  - Act: one Sigmoid instruction over the 3-gate PSUM tile [128, 3, TC].
  - DVE: fused custom op f = sa*~recip(sa+sb) (7 ALU stages, one pass);
    g = (pc + 0.5) max sg (scalar_tensor_tensor); tensor_tensor_scan.
  - Pool: v = (1-f)*g multiply; most of w = 1-f (ones - f subtract).
  - A fraction of w ops go to Act (Copy, scale=-1, bias=1) to balance.
Outputs stored f16 (DMA halved), upcast on host.
"""

from contextlib import ExitStack

import numpy as np
import ml_dtypes

import concourse.bacc as bacc
import concourse.tile as tile
import concourse.mybir as mybir
from concourse.bass_utils import run_bass_kernel_spmd

import concourse.dve_ops as _dve_ops
from concourse.dve_spec import (Spec as _Spec, Src0 as _S0, Src1 as _S1,
                                C1 as _C1, C2 as _C2,
                                AluOp as _AluOp, Bin as _Bin, lower as _lower)
from concourse.dve_uop import DveOpSpec as _DveOpSpec
from concourse.dve_table_gen import dve_ver_for as _dve_ver_for

# ---- fused custom DVE op: f = in0 * ~1/(in0+in1) ---------------------------
# x = in0+in1; nx = bitcast(~x) (exponent-flip reciprocal seed); u = x*nx
# lands in [-4.5,-4]; f = in0 * nx * (c1*u + c2), deg-1 minimax of 1/u on
# that interval (max rel err ~2.2e-3). 7 ALU stages -> single DVE pass.
GATE_FRAC_CONSTS = {"s1": -0.0546648, "imm2": -0.4681172}


def _register_gate_frac():
    name = "GATE_FRAC_ANT"
    if name in _dve_ops._SUB_OPCODE_FOR_NAME:
        return next(o for o in _dve_ops.OPS if o.name == name)
    _x = _S0 + _S1
    _nx = _Bin(_AluOp.BITWISE_NOT, _x, _x)
    _u = _x * _nx

    def _ref(in0, in1, c0, c1, c2):
        in0 = np.asarray(in0, np.float32)
        in1 = np.asarray(in1, np.float32)
        x = (in0 + in1).astype(np.float32)
        nx = (~x.view(np.int32)).view(np.float32)
        u = x * nx
        return (nx * (c1 * u + c2)) * in0

    spec = _Spec(body=(_Bin(_AluOp.MULTIPLY, _nx, (_C1 * _u + _C2))) * _S0,
                 reference=_ref)
    row = _dve_ops._CUSTOM_DVE_ROW_BASE + len(_dve_ops.OPS)
    assert row < 0x20
    ver = _dve_ver_for("TRN2")
    sha = _DveOpSpec(name=name, opcode=row, uops=_lower(spec, ver=ver),
                     rd1_en=True).sha(ver)
    op = _dve_ops.DveOp(name, spec, subdim=False, uops_sha={ver: sha})
    _dve_ops.OPS.append(op)
    _dve_ops.CUSTOM_DVE_SPECS[name] = spec
    _dve_ops._SUB_OPCODE_FOR_NAME[name] = row
    return op


_GATE_FRAC_OP = _register_gate_frac()

F32 = mybir.dt.float32
F16 = mybir.dt.float16
F8 = mybir.dt.float8e4
AF = mybir.ActivationFunctionType
OP = mybir.AluOpType
PM = mybir.MatmulPerfMode
E4 = ml_dtypes.float8_e4m3fn

B, T, D, H = 4, 8192, 512, 512
NCORES = 8
HS = H // 2          # 256 channels per core
TC = 512             # T chunk width
NCH = T // TC        # 16 chunks
NHT = HS // 128      # 2 h-tiles per core
NPAIR = 2            # DoubleRow k-tile pairs covering D=512

MM_MODE = "fp8dr"    # kept for test.py compatibility

# every W_ACT_EVERY-th (chunk,ht) unit computes w = 1-f on Act instead of
# Pool, balancing Act ~53us / Pool ~60us under the DVE ~63us bottleneck
W_ACT_EVERY = 1   # unused, kept for reference
# w-engine pattern: p=Pool, d=DVE, a=Act (balances the three engines)
W_PATTERN = "pap"

_nc_cache = {}


def _build_nc(mm_mode=MM_MODE):
    nc = bacc.Bacc("TRN2", target_bir_lowering=False, debug=False,
                   num_devices=NCORES)
    # x8 (slots 0-3) and xlo8 (slots 4-7) k-tiles, chunk-major
    xq = nc.dram_tensor("xq", [NCH * 128, 8, TC], F8, kind="ExternalInput")
    w8 = nc.dram_tensor("w8", [128, NPAIR, 2, 3 * HS], F8,
                        kind="ExternalInput")
    wlo = nc.dram_tensor("wlo", [128, NPAIR, 2, HS], F8,
                         kind="ExternalInput")
    aux = nc.dram_tensor("aux", [128, NHT], F32, kind="ExternalInput")
    out = nc.dram_tensor("out", [128, NHT, T], F16, kind="ExternalOutput")

    with tile.TileContext(nc) as tc, ExitStack() as ctx:
        wpool = ctx.enter_context(tc.tile_pool(name="w", bufs=1))
        xpool = ctx.enter_context(tc.tile_pool(name="x", bufs=4))
        gpool = ctx.enter_context(tc.tile_pool(name="g", bufs=6))
        hpool = ctx.enter_context(tc.tile_pool(name="h", bufs=5))
        ppool = ctx.enter_context(tc.tile_pool(name="p", bufs=2, space="PSUM"))

        # weight/aux loads on the ACT HWDGE queue so the first x-chunk loads
        # (SP queue) run in parallel with them
        wt = wpool.tile([128, NPAIR, 2, 3 * HS], F8, tag="w8", name="wt")
        nc.scalar.dma_start(wt[:], w8[:])
        wlt = wpool.tile([128, NPAIR, 2, HS], F8, tag="wlo", name="wlt")
        nc.scalar.dma_start(wlt[:], wlo[:])
        auxt = wpool.tile([128, NHT], F32, tag="aux", name="auxt")
        nc.scalar.dma_start(auxt[:], aux[:])
        ones = wpool.tile([128, TC], F16, tag="ones", name="ones")
        nc.vector.memset(ones[:], 1.0)

        # trigger the Act function-table loads immediately (they otherwise
        # land right before the first sigmoid and delay it)
        dummy = wpool.tile([128, 1], F16, tag="dummy", name="dummy")
        nc.scalar.activation(dummy[:], ones[:, 0:1], AF.Sigmoid)
        nc.scalar.activation(dummy[:], ones[:, 0:1], AF.Copy, bias=1.0,
                             scale=-1.0)

        # PE p-state warmup: the tensor engine needs ~3us of continuous work
        # to reach full clock. The first ~5us of the kernel are DMA-only, so
        # burn them on scratch matmuls (no data dependencies) and the first
        # real matmuls run at full speed.
        wu_s = wpool.tile([128, 2, 128], F8, tag="wu_s", name="wu_s")
        nc.vector.memset(wu_s[:], 0.0)
        wu_m = wpool.tile([128, 2, TC], F8, tag="wu_m", name="wu_m")
        nc.vector.memset(wu_m[:], 0.0)
        wu_p = ppool.tile([128, TC], F32, tag="Pc", name="wu_p", bufs=4)
        for _ in range(12):
            nc.tensor.matmul(wu_p[:], wu_s[:], wu_m[:], start=True,
                             stop=True, perf_mode=PM.DoubleRow)

        # chunk 0 split in half so the pipeline primes on half-size units
        chunks = [(0, TC // 2), (TC // 2, TC // 2)]
        chunks += [(k * TC, TC) for k in range(1, NCH)]

        NUNITS = len(chunks) * NHT
        carry = [None] * NHT
        it = 0
        # two-unit software pipeline: unit k's head [matmuls, sigmoid, g, f]
        # is emitted before unit k-2's tail [w, v, scan, out-dma]. The scan
        # of unit k-2 then never stalls the in-order DVE: its v input has
        # been computable since unit k-1's head ran.
        TAIL_DELAY = 2
        from collections import deque
        pendq = deque()  # (f, g, h_slice, ini, carry_ap, ht, ci, tw, unit#)
        out_dmas = {}    # ci -> (ht_tile, tsl, tw) emitted when ht=1 scanned

        def flush_pending(force=False):
            while pendq and (force or len(pendq) > TAIL_DELAY):
                f_, g_, h_, carry_ap, ht_, ci_, tw_, k_ = pendq.popleft()
                # resolve the scan initial lazily: the same-ht predecessor's
                # scan is emitted exactly TAIL_DELAY units before this flush
                ini_ = auxt[:, ht_:ht_ + 1] if ci_ == 0 else carry[ht_]
                # w = 1-f: Pool / DVE / Act per balance pattern; in the
                # drain (Act idle, Pool the laggard) w goes to Act and v to
                # DVE so the last scans issue as early as possible
                drain = k_ >= NUNITS - 2
                w_ = gpool.tile([128, TC], F16, tag="w", name="w_")[:, :tw_]
                wsel = "a" if drain else W_PATTERN[k_ % len(W_PATTERN)]
                if wsel == "a":
                    nc.scalar.activation(w_, f_, AF.Copy, bias=1.0,
                                         scale=-1.0)
                elif wsel == "d":
                    nc.vector.tensor_scalar(w_, f_, -1.0, 1.0,
                                            OP.mult, OP.add)
                else:
                    nc.gpsimd.tensor_tensor(w_, ones[:, :tw_], f_,
                                            op=OP.subtract)
                v_ = gpool.tile([128, TC], F16, tag="v", name="v_")[:, :tw_]
                if drain:
                    nc.vector.tensor_tensor(v_, w_, g_, op=OP.mult)
                else:
                    nc.gpsimd.tensor_tensor(v_, w_, g_, op=OP.mult)
                nc.vector.tensor_tensor_scan(h_, f_, v_, ini_, OP.mult,
                                             OP.add)
                carry[ht_] = carry_ap
                if ci_ == len(chunks) - 1:
                    htile_, tsl_, twc_ = out_dmas[ci_]
                    nc.sync.dma_start(out[:, ht_:ht_ + 1, tsl_],
                                      htile_[:, ht_:ht_ + 1, :twc_])
                elif ht_ == NHT - 1:
                    htile_, tsl_, twc_ = out_dmas.pop(ci_)
                    nc.sync.dma_start(out[:, :, tsl_], htile_[:, :, :twc_])

        for ci, (t0, tw) in enumerate(chunks):
            tsl = slice(t0, t0 + tw)
            xall = xpool.tile([128, 8, TC], F8, tag="xall", name="xall")
            crow = (t0 // TC) * 128
            csl = slice(t0 % TC, t0 % TC + tw)
            if ci == 0:
                # first chunk: land the x8 k-tiles (slots 0-3) before the
                # xlo slots so the f/i matmuls can start sooner
                nc.sync.dma_start(xall[:, :4, csl.start:csl.stop],
                                  xq[crow:crow + 128, :4, csl])
                nc.sync.dma_start(xall[:, 4:, csl.start:csl.stop],
                                  xq[crow:crow + 128, 4:, csl])
            else:
                nc.sync.dma_start(xall[:, :, csl.start:csl.stop] if tw != TC
                                  else xall[:],
                                  xq[crow:crow + 128, :, csl])
            ht_tile = hpool.tile([128, NHT, TC], F16, tag="h", name="ht_tile")
            out_dmas[ci] = (ht_tile, tsl, tw)
            for ht in range(NHT):
                # split PSUM: Pc (1 bank, 4-deep rotation — its late reader
                # g never gates the PE) and Pab (2 banks, 2-deep — freed by
                # sigmoid-ab alone)
                Pc = ppool.tile([128, TC], F32, tag="Pc", name="Pc", bufs=4)
                Pab = ppool.tile([128, 2, TC], F32, tag="Pab", name="Pab",
                                 bufs=2)
                for gate in range(3):
                    c0 = gate * HS + ht * 128
                    dst = Pc[:, :tw] if gate == 2 else Pab[:, gate, :tw]
                    nterm = 3 if gate == 2 else 1
                    step = 0
                    for term in range(nterm):
                        for p in range(NPAIR):
                            if term == 1:       # x8 @ Wlo8 (c gate only)
                                stat = wlt[:, p, :, ht * 128:ht * 128 + 128]
                                mov = xall[:, 2 * p:2 * p + 2, csl]
                            elif term == 2:     # xlo8 @ W8 (c gate only)
                                stat = wt[:, p, :, c0:c0 + 128]
                                mov = xall[:, 4 + 2 * p:4 + 2 * p + 2, csl]
                            else:               # x8 @ W8
                                stat = wt[:, p, :, c0:c0 + 128]
                                mov = xall[:, 2 * p:2 * p + 2, csl]
                            nc.tensor.matmul(
                                dst, stat, mov,
                                start=(step == 0),
                                stop=(step == 2 * nterm - 1),
                                perf_mode=PM.DoubleRow)
                            step += 1
                # sigmoid-c first (g on DVE consumes it), then sigmoid-ab
                sg = gpool.tile([128, TC], F16, tag="sg", name="sg")[:, :tw]
                nc.scalar.activation(sg, Pc[:, :tw], AF.Sigmoid)
                sab = gpool.tile([128, 2, TC], F16, tag="sab", name="sab")
                nc.scalar.activation(sab[:, :, :tw], Pab[:, :, :tw],
                                     AF.Sigmoid)
                # g = (pc + 0.5) max sg on DVE; last reader of Pc
                g = gpool.tile([128, TC], F16, tag="g", name="g")[:, :tw]
                nc.vector.scalar_tensor_tensor(g, Pc[:, :tw], 0.5, sg,
                                               OP.add, OP.max)
                # f = sa/(sa+sb) fused on DVE
                f = gpool.tile([128, TC], F16, tag="f", name="f")[:, :tw]
                nc.vector._custom_dve(_GATE_FRAC_OP, out=f,
                                      in0=sab[:, 0, :tw], in1=sab[:, 1, :tw],
                                      s0=0.0, s1=GATE_FRAC_CONSTS["s1"],
                                      imm2=GATE_FRAC_CONSTS["imm2"])
                # emit the PREVIOUS unit's tail (w, v, scan, out-dma) now
                # that this unit's head is queued ahead of it
                flush_pending()
                h = ht_tile[:, ht, :tw]
                carry_ap = ht_tile[:, ht, tw - 1:tw]
                pendq.append((f, g, h, carry_ap, ht, ci, tw, it))
                it += 1
        flush_pending(force=True)
    nc.compile()
    return nc


def _get_nc(mm_mode=MM_MODE):
    if mm_mode not in _nc_cache:
        _nc_cache[mm_mode] = _build_nc(mm_mode)
    return _nc_cache[mm_mode]


def _g_host(x):
    # exp(log_g(x)) of the reference, computed directly in fp32
    return np.where(x >= 0, x + 0.5, 1.0 / (1.0 + np.exp(-np.minimum(x, 0))))


def _prep_x(xb):
    """xb [T, D] f32 -> [NCH*128, 8, TC] e4m3 (x8 + xlo8 k-tiles)."""
    xt = np.ascontiguousarray(xb.reshape(NCH, TC, D).transpose(0, 2, 1))
    x8 = xt.astype(E4)                                   # [NCH, D, TC]
    xlo8 = (xt - x8.astype(np.float32)).astype(E4)
    # [NCH, 8, 128, TC] slot-major -> [NCH, 128, 8, TC]
    both = np.concatenate([x8.reshape(NCH, 4, 128, TC),
                           xlo8.reshape(NCH, 4, 128, TC)], axis=1)
    return np.ascontiguousarray(both.transpose(0, 2, 1, 3)).reshape(
        NCH * 128, 8, TC)


def _prep_w(wcat):
    """[D, C] f32 -> ([128, NPAIR, 2, C] e4m3 base, same-shape lo residual
    of the last HS columns)."""
    w8 = wcat.astype(E4)
    dev = np.ascontiguousarray(
        w8.reshape(NPAIR, 2, 128, wcat.shape[1]).transpose(2, 0, 1, 3))
    return dev


def _run(inputs, mm_mode=MM_MODE, trace=False):
    x = np.asarray(inputs["x"], np.float32)
    h_0 = np.asarray(inputs["h_0"], np.float32)
    W_f = np.asarray(inputs["W_f"], np.float32)
    b_f = np.asarray(inputs["b_f"], np.float32)
    W_i = np.asarray(inputs["W_i"], np.float32)
    b_i = np.asarray(inputs["b_i"], np.float32)
    W_h = np.asarray(inputs["W_h"], np.float32)
    b_h = np.asarray(inputs["b_h"], np.float32)
    assert (b_f == 0).all() and (b_i == 0).all() and (b_h == 0).all(), \
        "device program folds zero biases"

    g0 = _g_host(h_0[:, 0, :])  # [B, H]
    xqs = [_prep_x(x[b]) for b in range(B)]

    in_maps = []
    for c in range(NCORES):
        b, hh = divmod(c, 2)
        hs = slice(hh * HS, (hh + 1) * HS)
        wcat = np.concatenate([W_f[:, hs], W_i[:, hs], W_h[:, hs]], axis=1)
        w8dev = _prep_w(wcat)
        wh = W_h[:, hs]
        whlo = (wh - wh.astype(E4).astype(np.float32))
        wlodev = np.ascontiguousarray(
            whlo.astype(E4).reshape(NPAIR, 2, 128, HS).transpose(2, 0, 1, 3))
        auxa = np.ascontiguousarray(
            g0[b, hs].reshape(NHT, 128).T.astype(np.float32))
        in_maps.append({"xq": xqs[b], "w8": w8dev, "wlo": wlodev,
                        "aux": auxa})

    nc = _get_nc(mm_mode)
    res = run_bass_kernel_spmd(nc, in_maps, core_ids=list(range(NCORES)),
                               trace=trace)

    out = np.empty((B, T + 1, H), np.float32)
    out[:, 0, :] = g0
    for c in range(NCORES):
        b, hh = divmod(c, 2)
        hs = slice(hh * HS, (hh + 1) * HS)
        blk = res.results[c]["out"].astype(np.float32)   # [128, NHT, T]
        out[b, 1:, hs] = blk.transpose(2, 1, 0).reshape(T, HS)
    return out, res


def kernel(**inputs):
    out, _ = _run(inputs)
    return out
